# revision 10
# baseline (speedup 1.0000x reference)
"""Trainium2 Bass kernel for nn_LossComputation (multi-loss: instance CE,
global-align, CMPC CE, CMPM KL, max-hinge) on 8 NeuronCores.

Sharding: the classifier dim C (=11003, padded to 8*1376) of W/W2 is sharded
across the 8 cores for the four big [B,D]@[D,C] matmuls + CE partials; the
batch dim is sharded across cores for the BxB similarity losses. All
cross-core reductions are tiny [B]-vectors combined on the host.
"""
import sys

if "/opt/trn_rl_repo" not in sys.path:
    sys.path.insert(0, "/opt/trn_rl_repo")

import contextlib

import numpy as np

import concourse.bass as bass
import concourse.mybir as mybir
import concourse.tile as tile
from concourse.masks import make_identity
from concourse.vector_clock import ScopedClock, VectorClock
from concourse.bass_utils import run_bass_kernel_spmd

F32 = mybir.dt.float32
AF = mybir.ActivationFunctionType
ALU = mybir.AluOpType
AX = mybir.AxisListType

SCALE = 28.0
MARGIN = 0.2
ALPHA, BETA = 0.6, 0.4
SCALE_POS, SCALE_NEG = 10.0, 40.0
EPS = 1e-8
LOG_EPS = float(np.log(np.float32(EPS)))

B, D, C = 1024, 1024, 11003
NCORES = 8
CS = 1376               # per-core padded shard width (8*1376 = 11008)
NPAD = NCORES * CS - C  # 5 pad columns, all on core 7
ROWS = B // NCORES      # 128 batch rows per core for the BxB losses
NT = 8                  # number of 128-row/128-col tiles in B or D
CHUNKS = [(0, 512), (512, 512), (1024, 352)]  # n-chunks of the C shard

_NC_CACHE = {}


# Tile emits one final drain carrying one sem-wait per logical proc; this
# walrus build rejects >2 sync waits on an SP CTRL instruction, so emit one
# drain per proc instead.
def _patched_drain_and_barrier(self, tick_clock, wait_clock):
    ticks = list(tick_clock.global_clock)
    n = len(ticks)
    for i in range(n):
        if ticks[i] == 0:
            continue
        sub = [0] * n
        sub[i] = ticks[i]
        d = self.nc.sync.drain()
        wait_clock.add_sem_waits(d.ins, ScopedClock({None: VectorClock(sub)}))
    self.nc.all_engine_barrier()
    assert self.sems is not None
    popped = self.nc._tile_sem_poison_stack.pop()
    assert popped is self._sem_poison
    self.nc.clear_and_free_semaphores(list(self.sems.allocated().values()))
    self.nc.all_engine_barrier()


def _split_excess_waits(nc, maxw=2):
    """Walrus in this build rejects instructions carrying more than `maxw`
    sync-wait commands. Split the excess onto fresh NoOps inserted right
    before the offending instruction on the same engine — the engine is
    in-order, so waiting across two adjacent instruction slots is
    equivalent to waiting on one."""
    n_split = 0
    for f in nc.m.functions:
        for b in f.blocks:
            lst = b.instructions
            i = 0
            while i < len(lst):
                ins = lst[i]
                si = ins.sync_info
                waits = list(si.on_wait) if si and si.on_wait else []
                if len(waits) > maxw:
                    keep = waits[-maxw:]
                    excess = waits[:-maxw]
                    si.on_wait = keep
                    pos = i
                    for j in range(0, len(excess), maxw):
                        chunk = excess[j : j + maxw]
                        nop = mybir.InstNoOp(
                            name=f"{ins.name}-wsplit{j}", ins=[], outs=[])
                        nop.engine = ins.engine
                        nop.sync_info = mybir.SyncInfo(on_wait=chunk,
                                                       on_update=[])
                        lst.insert(pos, nop)
                        pos += 1
                        n_split += 1
                    i = pos
                i += 1
    return n_split


def _apply_patches():
    tile.TileContext._drain_and_barrier = _patched_drain_and_barrier
    # The container's antenv package lacks axon_hooks; shim it so the
    # trace=True path (NTFF profiling) works instead of crashing, and skip
    # the cloud artifact upload which isn't available here.
    try:
        import types

        import antenv
        from concourse import bass_utils as _bu

        if "antenv.axon_hooks" not in sys.modules:
            hooks = types.ModuleType("antenv.axon_hooks")
            hooks._hook = None
            hooks.set_axon_ntff_profile_hook = lambda h: setattr(hooks, "_hook", h)
            hooks.get_axon_ntff_profile_hook = lambda: hooks._hook
            sys.modules["antenv.axon_hooks"] = hooks
            antenv.axon_hooks = hooks
            try:
                from trn_agent_boot.trn_boot import _ntff_profile_via_ctypes

                hooks._hook = _ntff_profile_via_ctypes("/opt/axon/libaxon_pjrt.so")
            except Exception:
                pass
        _bu.upload_artifacts = lambda tmpdir: "/tmp/bass_artifacts_noupload"
    except Exception:
        pass


def _bcast_row_ap(dram_t, width, parts=128):
    """AP reading a [1, width] DRAM row broadcast over `parts` partitions."""
    ap = dram_t.ap()
    return bass.AP(tensor=ap.tensor, offset=ap.offset, ap=[[0, parts], [1, width]])


def build_nc():
    nc = bass.Bass("TRN2", target_bir_lowering=False, debug=False,
                   num_devices=NCORES)

    # register activation bias constants (only 0.0/1.0 are pre-registered)
    for val in (-SCALE_NEG * BETA, SCALE_POS * ALPHA, MARGIN):
        ct = nc.alloc_sbuf_tensor(f"const-float32-{val}", [128, 1], F32)
        nc.gpsimd.memset(ct.ap(), val)
        nc.const_aps.aps[(F32, float(val))] = ct.ap()
    nc.all_engine_barrier()

    din = {}

    def dram_in(name, shape):
        din[name] = nc.dram_tensor(name, shape, F32, kind="ExternalInput")
        return din[name]

    dout = {}

    def dram_out(name, shape):
        dout[name] = nc.dram_tensor(name, shape, F32, kind="ExternalOutput")
        return dout[name]

    v_d = dram_in("v", [B, D])
    t_d = dram_in("t", [B, D])
    w_d = dram_in("w", [D, CS])
    w2_d = dram_in("w2", [D, CS])
    padmask_d = dram_in("padmask", [1, CS])
    onehot_d = dram_in("onehot", [B, 256])
    labrow_d = dram_in("labrow", [1, B])
    logmn_d = dram_in("logmn", [1, B])
    lab_local_d = dram_in("lab_local", [ROWS, 1])
    vloc_d = dram_in("vloc", [ROWS, D])
    tloc_d = dram_in("tloc", [ROWS, D])

    for nm in ("S_v", "S_t", "lab_v", "lab_t",
               "S2_v", "S2_t", "lab2_v", "lab2_t",
               "labE2_v", "labE2_t", "M2E_v", "M2E_t", "mh_im"):
        dram_out(nm, [B, 1])
    for nm in ("ga_n", "ga_m", "mh_s", "i2t", "t2i"):
        dram_out(nm, [ROWS, 1])

    v_tiled = v_d.ap().rearrange("(n p) d -> p n d", p=128)
    t_tiled = t_d.ap().rearrange("(n p) d -> p n d", p=128)
    w_tiled = w_d.ap().rearrange("(k p) c -> p k c", p=128)
    w2_tiled = w2_d.ap().rearrange("(k p) c -> p k c", p=128)

    with tile.TileContext(nc) as tc, contextlib.ExitStack() as ctx:
        persist = ctx.enter_context(tc.tile_pool(name="persist", bufs=1))
        ident = persist.tile([128, 128], F32)
        make_identity(nc, ident)
        ones_col = persist.tile([128, 1], F32)
        nc.vector.memset(ones_col, 1.0)
        ones_row = persist.tile([1, 128], F32)
        nc.vector.memset(ones_row, 1.0)

        def bcast_row(row_tile, out_tile, width):
            """[1, width] -> [128, width] via K=1 PE matmul + PSUM evict."""
            for lo in range(0, width, 512):
                nsz = min(512, width - lo)
                pb = psum_tr.tile([128, 512], F32, tag="tr")
                nc.tensor.matmul(pb[:, :nsz], lhsT=ones_row,
                                 rhs=row_tile[:, lo : lo + nsz],
                                 start=True, stop=True)
                nc.scalar.copy(out=out_tile[:, lo : lo + nsz], in_=pb[:, :nsz])

        vnT = persist.tile([128, NT, B], F32)   # vn^T as k-tiles of D
        tnT = persist.tile([128, NT, B], F32)
        rv_s = persist.tile([128, NT], F32)     # rowsum(v^2), col n = row-tile
        rt_s = persist.tile([128, NT], F32)
        riv_s = persist.tile([128, NT], F32)    # 1/||v_b||
        rit_s = persist.tile([128, NT], F32)
        d_s = persist.tile([128, NT], F32)      # diag(sim)
        p_s = persist.tile([128, NT], F32)      # v.tn = d*||v||
        q_s = persist.tile([128, NT], F32)      # t.vn = d*||t||
        scr8 = persist.tile([128, NT], F32)

        pool_w = ctx.enter_context(tc.tile_pool(name="pool_w", bufs=1))
        w_sb = pool_w.tile([128, NT, CS], F32, tag="w")
        nc.sync.dma_start(out=w_sb, in_=w_tiled)

        pool_st = ctx.enter_context(tc.tile_pool(name="pool_st", bufs=2))

        psum_mm = ctx.enter_context(
            tc.tile_pool(name="psum_mm", bufs=4, space="PSUM"))
        psum_tr = ctx.enter_context(
            tc.tile_pool(name="psum_tr", bufs=2, space="PSUM"))
        psum_row = ctx.enter_context(
            tc.tile_pool(name="psum_row", bufs=2, space="PSUM"))

        # ---------------- Phase 0: norms + transposes ----------------
        with tc.tile_pool(name="pool_p0", bufs=1) as pool_p0:
            vfull = pool_p0.tile([128, NT, D], F32, tag="vf")
            tfull = pool_p0.tile([128, NT, D], F32, tag="tf")
            nc.sync.dma_start(out=vfull, in_=v_tiled)
            nc.sync.dma_start(out=tfull, in_=t_tiled)

            for n in range(NT):
                sq = pool_st.tile([128, D], F32, tag="p0sq")
                nc.scalar.activation(out=sq, in_=vfull[:, n, :], func=AF.Square,
                                     accum_out=rv_s[:, n : n + 1])
                sq2 = pool_st.tile([128, D], F32, tag="p0sq")
                nc.scalar.activation(out=sq2, in_=tfull[:, n, :], func=AF.Square,
                                     accum_out=rt_s[:, n : n + 1])
            nc.scalar.activation(out=scr8, in_=rv_s, func=AF.Sqrt)
            nc.vector.reciprocal(out=riv_s, in_=scr8)
            nc.scalar.activation(out=scr8, in_=rt_s, func=AF.Sqrt)
            nc.vector.reciprocal(out=rit_s, in_=scr8)

            for n in range(NT):
                nc.vector.tensor_scalar_mul(vfull[:, n, :], vfull[:, n, :],
                                            riv_s[:, n : n + 1])
                nc.vector.tensor_scalar_mul(tfull[:, n, :], tfull[:, n, :],
                                            rit_s[:, n : n + 1])
                dsc = pool_st.tile([128, D], F32, tag="p0sq")
                nc.vector.scalar_tensor_tensor(
                    out=dsc, in0=vfull[:, n, :], scalar=1.0,
                    in1=tfull[:, n, :], op0=ALU.mult, op1=ALU.mult,
                    accum_out=d_s[:, n : n + 1])

            # p = d*||v|| = d*rv*riv ; q = d*||t||
            nc.vector.tensor_tensor(out=scr8, in0=rv_s, in1=riv_s, op=ALU.mult)
            nc.vector.tensor_tensor(out=p_s, in0=d_s, in1=scr8, op=ALU.mult)
            nc.vector.tensor_tensor(out=scr8, in0=rt_s, in1=rit_s, op=ALU.mult)
            nc.vector.tensor_tensor(out=q_s, in0=d_s, in1=scr8, op=ALU.mult)

            # transposes: batch 4 row-tiles n per PSUM bank, evict 512 at once
            for src, dst in ((vfull, vnT), (tfull, tnT)):
                for m in range(NT):
                    for g in range(2):
                        ptile = psum_tr.tile([128, 512], F32, tag="tr")
                        for j in range(4):
                            n = 4 * g + j
                            nc.tensor.transpose(
                                out=ptile[:, 128 * j : 128 * (j + 1)],
                                in_=src[:, n, 128 * m : 128 * (m + 1)],
                                identity=ident)
                        nc.scalar.copy(out=dst[:, m, 512 * g : 512 * (g + 1)],
                                       in_=ptile)

        # ---------------- main pool for phases A/C/B ----------------
        pool_ac = ctx.enter_context(tc.tile_pool(name="pool_ac", bufs=1))

        def col_scale_W(wsb, scale_const):
            """colssq -> wsc row -> broadcast -> scale W columns in place."""
            cs_row = pool_ac.tile([1, CS], F32, tag="rowA")
            for ci, (nlo, nsz) in enumerate(CHUNKS):
                pr = psum_row.tile([1, 512], F32, tag="vrow")
                for k in range(NT):
                    sq = pool_st.tile([128, 512], F32, tag="wsq")
                    nc.scalar.activation(out=sq[:, :nsz],
                                         in_=wsb[:, k, nlo : nlo + nsz],
                                         func=AF.Square)
                    nc.tensor.matmul(pr[:, :nsz], lhsT=ones_col,
                                     rhs=sq[:, :nsz],
                                     start=(k == 0), stop=(k == NT - 1))
                nc.scalar.activation(out=cs_row[:, nlo : nlo + nsz],
                                     in_=pr[:, :nsz], func=AF.Sqrt)
            nc.vector.reciprocal(out=cs_row, in_=cs_row)
            nc.vector.tensor_scalar_mul(cs_row, cs_row, float(scale_const))
            pm = pool_ac.tile([1, CS], F32, tag="rowC")
            nc.sync.dma_start(out=pm, in_=padmask_d.ap())
            nc.vector.tensor_tensor(out=cs_row, in0=cs_row, in1=pm, op=ALU.mult)
            wsc_bc = pool_ac.tile([128, CS], F32, tag="wsc_bc")
            bcast_row(cs_row, wsc_bc, CS)
            for k in range(NT):
                nc.gpsimd.tensor_tensor(out=wsb[:, k, :], in0=wsb[:, k, :],
                                        in1=wsc_bc, op=ALU.mult)

        def ce_branch(wsb, lhsT_sb, S_o, lab_o, labE_o=None, M2_o=None):
            """logits = lhsT^T @ wsb per m-tile; exp-sums + label logit
            (+ max-exp and label-exp when requested, for cmpc precision)."""
            for mt in range(NT):
                oh = pool_st.tile([128, 256], F32, tag="oh")
                nc.sync.dma_start(out=oh,
                                  in_=onehot_d.ap()[128 * mt : 128 * (mt + 1), :])
                sp = pool_st.tile([128, 4], F32, tag="sp")
                lab_col = pool_st.tile([128, 1], F32, tag="labc")
                if M2_o is not None:
                    mp = pool_st.tile([128, 4], F32, tag="mp")
                    labE_col = pool_st.tile([128, 1], F32, tag="labEc")
                for ci, (nlo, nsz) in enumerate(CHUNKS):
                    ps = psum_mm.tile([128, 512], F32, tag="mm")
                    for k in range(NT):
                        nc.tensor.matmul(
                            ps[:, :nsz],
                            lhsT=lhsT_sb[:, k, 128 * mt : 128 * (mt + 1)],
                            rhs=wsb[:, k, nlo : nlo + nsz],
                            start=(k == 0), stop=(k == NT - 1))
                    e = pool_st.tile([128, 512], F32, tag="e_scr")
                    nc.scalar.activation(out=e[:, :nsz], in_=ps[:, :nsz],
                                         func=AF.Exp,
                                         accum_out=sp[:, ci : ci + 1])
                    if ci == 0:
                        scr = pool_st.tile([128, 256], F32, tag="scr256")
                        nc.vector.scalar_tensor_tensor(
                            out=scr, in0=ps[:, 0:256], scalar=1.0, in1=oh,
                            op0=ALU.mult, op1=ALU.mult, accum_out=lab_col)
                        if labE_o is not None:
                            scr2 = pool_st.tile([128, 256], F32, tag="scr256")
                            nc.vector.scalar_tensor_tensor(
                                out=scr2, in0=e[:, 0:256], scalar=1.0, in1=oh,
                                op0=ALU.mult, op1=ALU.mult, accum_out=labE_col)
                    if M2_o is not None:
                        nc.vector.reduce_max(out=mp[:, ci : ci + 1],
                                             in_=e[:, :nsz], axis=AX.X)
                rs = slice(128 * mt, 128 * (mt + 1))
                s_col = pool_st.tile([128, 1], F32, tag="scol")
                nc.vector.reduce_sum(out=s_col, in_=sp[:, 0:3], axis=AX.X)
                nc.sync.dma_start(out=S_o.ap()[rs, :], in_=s_col)
                nc.sync.dma_start(out=lab_o.ap()[rs, :], in_=lab_col)
                if M2_o is not None:
                    m_col = pool_st.tile([128, 1], F32, tag="mcol")
                    nc.vector.reduce_max(out=m_col, in_=mp[:, 0:3], axis=AX.X)
                    nc.sync.dma_start(out=M2_o.ap()[rs, :], in_=m_col)
                    nc.sync.dma_start(out=labE_o.ap()[rs, :], in_=labE_col)

        # ---------------- Phase A: instance loss ----------------
        col_scale_W(w_sb, SCALE)
        ce_branch(w_sb, vnT, dout["S_v"], dout["lab_v"])
        ce_branch(w_sb, tnT, dout["S_t"], dout["lab_t"])

        # W2 reuses the same pool slot; its DMA waits for W's last reader
        w2_sb = pool_w.tile([128, NT, CS], F32, tag="w")
        nc.sync.dma_start(out=w2_sb, in_=w2_tiled)

        # ---------------- Phase C: BxB losses on the local row shard ----
        vloc = pool_ac.tile([128, D], F32, tag="cscA")
        tloc = pool_ac.tile([128, D], F32, tag="cscB")
        nc.sync.dma_start(out=vloc, in_=vloc_d.ap())
        nc.sync.dma_start(out=tloc, in_=tloc_d.ap())

        # local row stats: cols 0..7 = rvl, rtl, rivl, ritl, dloc, -, -, tmp
        vstat = pool_ac.tile([128, 8], F32, tag="vstat")
        sqv = pool_st.tile([128, D], F32, tag="p0sq")
        nc.scalar.activation(out=sqv, in_=vloc, func=AF.Square,
                             accum_out=vstat[:, 0:1])
        sqt = pool_st.tile([128, D], F32, tag="p0sq")
        nc.scalar.activation(out=sqt, in_=tloc, func=AF.Square,
                             accum_out=vstat[:, 1:2])
        nc.scalar.activation(out=vstat[:, 2:4], in_=vstat[:, 0:2], func=AF.Sqrt)
        nc.vector.reciprocal(out=vstat[:, 2:4], in_=vstat[:, 2:4])
        dsc = pool_st.tile([128, D], F32, tag="p0sq")
        nc.vector.scalar_tensor_tensor(
            out=dsc, in0=vloc, scalar=1.0, in1=tloc,
            op0=ALU.mult, op1=ALU.mult, accum_out=vstat[:, 4:5])
        nc.vector.tensor_tensor(out=vstat[:, 7:8], in0=vstat[:, 2:3],
                                in1=vstat[:, 3:4], op=ALU.mult)
        nc.vector.tensor_tensor(out=vstat[:, 4:5], in0=vstat[:, 4:5],
                                in1=vstat[:, 7:8], op=ALU.mult)

        vn_loc = pool_ac.tile([128, D], F32, tag="cscC")
        nc.vector.tensor_scalar_mul(vn_loc, vloc, vstat[:, 2:3])

        # transposes of vloc, tloc, vn_loc -> [128, k, 128] lhsT forms
        vT_loc = pool_ac.tile([128, NT, 128], F32, tag="vT_loc")
        tT_loc = pool_ac.tile([128, NT, 128], F32, tag="tT_loc")
        vnT_loc = pool_ac.tile([128, NT, 128], F32, tag="vnT_loc")
        for src, dst in ((vloc, vT_loc), (tloc, tT_loc), (vn_loc, vnT_loc)):
            for g in range(2):
                ptile = psum_tr.tile([128, 512], F32, tag="tr")
                for j in range(4):
                    k = 4 * g + j
                    nc.tensor.transpose(
                        out=ptile[:, 128 * j : 128 * (j + 1)],
                        in_=src[:, 128 * k : 128 * (k + 1)], identity=ident)
                for j in range(4):
                    nc.scalar.copy(out=dst[:, 4 * g + j, :],
                                   in_=ptile[:, 128 * j : 128 * (j + 1)])

        # mask / notmask / sel2
        labrow_bc = pool_ac.tile([128, B], F32, tag="bc1")
        nc.sync.dma_start(out=labrow_bc, in_=_bcast_row_ap(labrow_d, B))
        lab_loc = pool_ac.tile([128, 1], F32, tag="lab_loc")
        nc.sync.dma_start(out=lab_loc, in_=lab_local_d.ap())
        mask = pool_ac.tile([128, B], F32, tag="mask")
        nc.vector.tensor_scalar(out=mask, in0=labrow_bc, scalar1=lab_loc,
                                scalar2=None, op0=ALU.is_equal)
        notmask = pool_ac.tile([128, B], F32, tag="notmask")
        nc.vector.tensor_scalar(out=notmask, in0=mask, scalar1=0.0,
                                scalar2=None, op0=ALU.is_equal)
        logmn_bc = pool_ac.tile([128, B], F32, tag="bc1")
        nc.sync.dma_start(out=logmn_bc, in_=_bcast_row_ap(logmn_d, B))
        sel2 = pool_ac.tile([128, B], F32, tag="sel2")
        nc.vector.tensor_scalar(out=sel2, in0=logmn_bc, scalar1=LOG_EPS,
                                scalar2=None, op0=ALU.subtract)
        nc.vector.tensor_tensor(out=sel2, in0=sel2, in1=mask, op=ALU.mult)
        nc.vector.tensor_scalar(out=sel2, in0=sel2, scalar1=LOG_EPS,
                                scalar2=None, op0=ALU.add)

        # d row (full diag) -> broadcast
        drow = pool_ac.tile([1, B], F32, tag="rowD")
        for n in range(NT):
            pr = psum_row.tile([1, 512], F32, tag="vrow")
            nc.tensor.transpose(out=pr[:, 0:128], in_=d_s[:, n : n + 1],
                                identity=ident)
            nc.scalar.copy(out=drow[:, 128 * n : 128 * (n + 1)], in_=pr[:, 0:128])
        drow_bc = pool_ac.tile([128, B], F32, tag="bc3")
        bcast_row(drow, drow_bc, B)

        # sim = vn_loc @ tn^T  [128, B]
        sim_sb = pool_ac.tile([128, B], F32, tag="sim")
        for n2 in range(2):
            ps = psum_mm.tile([128, 512], F32, tag="mm")
            for k in range(NT):
                nc.tensor.matmul(ps, lhsT=vnT_loc[:, k, :],
                                 rhs=tnT[:, k, 512 * n2 : 512 * (n2 + 1)],
                                 start=(k == 0), stop=(k == NT - 1))
            nc.scalar.copy(out=sim_sb[:, 512 * n2 : 512 * (n2 + 1)], in_=ps)

        # ---- global-align ----
        ga_n_col = pool_st.tile([128, 1], F32, tag="ga_n")
        ga_m_col = pool_st.tile([128, 1], F32, tag="ga_m")
        e_neg = pool_ac.tile([128, B], F32, tag="cscA")
        nc.scalar.activation(out=e_neg, in_=sim_sb, func=AF.Exp,
                             scale=SCALE_NEG, bias=-SCALE_NEG * BETA)
        ln_neg = pool_ac.tile([128, B], F32, tag="cscB")
        nc.scalar.activation(out=ln_neg, in_=e_neg, func=AF.Ln, bias=1.0,
                             accum_out=ga_n_col)
        e_pos = pool_ac.tile([128, B], F32, tag="cscA")
        nc.scalar.activation(out=e_pos, in_=sim_sb, func=AF.Exp,
                             scale=-SCALE_POS, bias=SCALE_POS * ALPHA)
        ln_pos = pool_ac.tile([128, B], F32, tag="cscC")
        nc.scalar.activation(out=ln_pos, in_=e_pos, func=AF.Ln, bias=1.0)
        nc.vector.tensor_tensor(out=ln_pos, in0=ln_pos, in1=ln_neg,
                                op=ALU.subtract)
        scr = pool_ac.tile([128, B], F32, tag="cscB")
        nc.vector.scalar_tensor_tensor(
            out=scr, in0=ln_pos, scalar=1.0, in1=mask,
            op0=ALU.mult, op1=ALU.mult, accum_out=ga_m_col)
        nc.sync.dma_start(out=dout["ga_n"].ap(), in_=ga_n_col)
        nc.sync.dma_start(out=dout["ga_m"].ap(), in_=ga_m_col)

        # ---- max-hinge ----
        u = pool_ac.tile([128, B], F32, tag="cscA")
        nc.vector.tensor_scalar(out=u, in0=sim_sb, scalar1=vstat[:, 4:5],
                                scalar2=None, op0=ALU.subtract)
        r = pool_ac.tile([128, B], F32, tag="cscB")
        nc.scalar.activation(out=r, in_=u, func=AF.Relu, bias=MARGIN)
        mh_s_col = pool_st.tile([128, 1], F32, tag="mh_s")
        scr = pool_ac.tile([128, B], F32, tag="cscA")
        nc.vector.tensor_tensor(out=scr, in0=r, in1=notmask, op=ALU.mult)
        nc.vector.reduce_max(out=mh_s_col, in_=scr, axis=AX.X)
        nc.sync.dma_start(out=dout["mh_s"].ap(), in_=mh_s_col)

        u2 = pool_ac.tile([128, B], F32, tag="cscA")
        nc.vector.tensor_tensor(out=u2, in0=sim_sb, in1=drow_bc,
                                op=ALU.subtract)
        r2 = pool_ac.tile([128, B], F32, tag="cscB")
        nc.scalar.activation(out=r2, in_=u2, func=AF.Relu, bias=MARGIN)
        cim = pool_ac.tile([128, B], F32, tag="cscA")
        nc.vector.tensor_tensor(out=cim, in0=r2, in1=notmask, op=ALU.mult)
        for g in range(2):
            ptile = psum_tr.tile([128, 512], F32, tag="tr")
            for j in range(4):
                jt = 4 * g + j
                nc.tensor.transpose(out=ptile[:, 128 * j : 128 * (j + 1)],
                                    in_=cim[:, 128 * jt : 128 * (jt + 1)],
                                    identity=ident)
            for j in range(4):
                jt = 4 * g + j
                mh_col = pool_st.tile([128, 1], F32, tag="mhim")
                nc.vector.reduce_max(out=mh_col,
                                     in_=ptile[:, 128 * j : 128 * (j + 1)],
                                     axis=AX.X)
                nc.sync.dma_start(
                    out=dout["mh_im"].ap()[128 * jt : 128 * (jt + 1), :],
                    in_=mh_col)

        # ---- cmpm (i2t / t2i) ----
        def cmpm_branch(lhsT_loc, rhsT, out_name):
            Ecm = pool_ac.tile([128, B], F32, tag="cscA")
            t3 = pool_ac.tile([128, B], F32, tag="cscB")
            zpart = pool_st.tile([128, 2], F32, tag="zpart")
            for n2 in range(2):
                ps = psum_mm.tile([128, 512], F32, tag="mm")
                for k in range(NT):
                    nc.tensor.matmul(ps, lhsT=lhsT_loc[:, k, :],
                                     rhs=rhsT[:, k, 512 * n2 : 512 * (n2 + 1)],
                                     start=(k == 0), stop=(k == NT - 1))
                sl = slice(512 * n2, 512 * (n2 + 1))
                nc.scalar.activation(out=Ecm[:, sl], in_=ps, func=AF.Exp,
                                     accum_out=zpart[:, n2 : n2 + 1])
                nc.vector.tensor_tensor(out=t3[:, sl], in0=ps, in1=sel2[:, sl],
                                        op=ALU.subtract)
            Z = pool_st.tile([128, 1], F32, tag="Zc")
            nc.vector.reduce_sum(out=Z, in_=zpart, axis=AX.X)
            logZ = pool_st.tile([128, 1], F32, tag="logZc")
            nc.scalar.activation(out=logZ, in_=Z, func=AF.Ln)
            nc.vector.tensor_scalar(out=t3, in0=t3, scalar1=logZ,
                                    scalar2=None, op0=ALU.subtract)
            srow = pool_st.tile([128, 1], F32, tag="srow")
            scr = pool_ac.tile([128, B], F32, tag="cscC")
            nc.vector.scalar_tensor_tensor(
                out=scr, in0=t3, scalar=1.0, in1=Ecm,
                op0=ALU.mult, op1=ALU.mult, accum_out=srow)
            rz = pool_st.tile([128, 1], F32, tag="rzc")
            nc.vector.reciprocal(out=rz, in_=Z)
            ocol = pool_st.tile([128, 1], F32, tag="ocol")
            nc.vector.tensor_tensor(out=ocol, in0=srow, in1=rz, op=ALU.mult)
            nc.sync.dma_start(out=dout[out_name].ap(), in_=ocol)

        cmpm_branch(vT_loc, tnT, "i2t")
        cmpm_branch(tT_loc, vnT, "t2i")

        # ---------------- Phase B: cmpc ----------------
        col_scale_W(w2_sb, 1.0)

        # p/q rows -> broadcast
        prow = pool_ac.tile([1, B], F32, tag="rowD")
        qrow = pool_ac.tile([1, B], F32, tag="rowE")
        for stat, row in ((p_s, prow), (q_s, qrow)):
            for n in range(NT):
                pr = psum_row.tile([1, 512], F32, tag="vrow")
                nc.tensor.transpose(out=pr[:, 0:128], in_=stat[:, n : n + 1],
                                    identity=ident)
                nc.scalar.copy(out=row[:, 128 * n : 128 * (n + 1)],
                               in_=pr[:, 0:128])
        pbc = pool_ac.tile([128, B], F32, tag="bc1")
        bcast_row(prow, pbc, B)
        qbc = pool_ac.tile([128, B], F32, tag="bc3")
        bcast_row(qrow, qbc, B)

        # vptT = tnT * p (in place over tnT), then v-branch
        for k in range(NT):
            nc.vector.tensor_tensor(out=tnT[:, k, :], in0=tnT[:, k, :],
                                    in1=pbc, op=ALU.mult)
        ce_branch(w2_sb, tnT, dout["S2_v"], dout["lab2_v"],
                  labE_o=dout["labE2_v"], M2_o=dout["M2E_v"])
        # tpvT = vnT * q (in place over vnT), then t-branch
        for k in range(NT):
            nc.vector.tensor_tensor(out=vnT[:, k, :], in0=vnT[:, k, :],
                                    in1=qbc, op=ALU.mult)
        ce_branch(w2_sb, vnT, dout["S2_t"], dout["lab2_t"],
                  labE_o=dout["labE2_t"], M2_o=dout["M2E_t"])

    _split_excess_waits(nc, maxw=1)
    return nc


def _host_prep(inputs):
    v = np.ascontiguousarray(np.asarray(inputs["visual_embed"], np.float32))
    t = np.ascontiguousarray(np.asarray(inputs["textual_embed"], np.float32))
    W = np.asarray(inputs["W"], np.float32)
    W2 = np.asarray(inputs["W2"], np.float32)
    labels = np.asarray(inputs["labels"]).astype(np.int64)

    labf = labels.astype(np.float32)
    onehot = np.zeros((B, 256), np.float32)
    onehot[np.arange(B), labels] = 1.0
    counts = np.bincount(labels, minlength=256)[labels]
    norm = np.sqrt(counts.astype(np.float32))
    logmn = np.log((np.float32(1.0) / norm).astype(np.float32)
                   + np.float32(EPS)).astype(np.float32)

    pad = np.ones((D, NPAD), np.float32)
    in_maps = []
    for k in range(NCORES):
        lo = k * CS
        hi = min((k + 1) * CS, C)
        wk = W[:, lo:hi]
        w2k = W2[:, lo:hi]
        pmk = np.ones((1, CS), np.float32)
        if hi - lo < CS:
            wk = np.concatenate([wk, pad[:, : CS - (hi - lo)]], axis=1)
            w2k = np.concatenate([w2k, pad[:, : CS - (hi - lo)]], axis=1)
            pmk[0, hi - lo :] = 0.0
        rs = slice(k * ROWS, (k + 1) * ROWS)
        in_maps.append({
            "v": v, "t": t,
            "w": np.ascontiguousarray(wk),
            "w2": np.ascontiguousarray(w2k),
            "padmask": pmk,
            "onehot": onehot,
            "labrow": labf[None, :].copy(),
            "logmn": logmn[None, :].copy(),
            "lab_local": np.ascontiguousarray(labf[rs, None]),
            "vloc": np.ascontiguousarray(v[rs]),
            "tloc": np.ascontiguousarray(t[rs]),
        })
    return in_maps


def _host_combine(outs):
    f32 = np.float32

    def ce(Ss, labs):
        S = np.sum([o[Ss][:, 0] for o in outs], axis=0) - NPAD
        return np.log(S).mean() - outs[0][labs][:, 0].mean()

    instance = ce("S_v", "lab_v") + ce("S_t", "lab_t")
    cmpc = ce("S2_v", "lab2_v") + ce("S2_t", "lab2_t")

    M2g_v = np.max([o["M2E_v"][:, 0] for o in outs], axis=0)
    M2g_t = np.max([o["M2E_t"][:, 0] for o in outs], axis=0)
    prec_v = (outs[0]["labE2_v"][:, 0] == M2g_v).mean()
    prec_t = (outs[0]["labE2_t"][:, 0] == M2g_t).mean()

    ga = 2.0 * np.sum([o["ga_n"].sum() + o["ga_m"].sum() for o in outs]) / B
    cmpm = np.concatenate([o["i2t"][:, 0] for o in outs]).mean() + \
        np.concatenate([o["t2i"][:, 0] for o in outs]).mean()
    mh = np.sum([o["mh_s"].sum() for o in outs]) + \
        np.max([o["mh_im"][:, 0] for o in outs], axis=0).sum()

    losses = np.array([instance, ga, cmpc, cmpm, mh], f32)
    precs = np.array([prec_v, prec_t], f32)
    return losses, precs


def run_cores(inputs, trace=False):
    """Build+compile (cached), run on the 8 cores, return (res, combined)."""
    _apply_patches()
    if "nc" not in _NC_CACHE:
        _NC_CACHE["nc"] = build_nc()
    nc = _NC_CACHE["nc"]
    in_maps = _host_prep(inputs)
    res = run_bass_kernel_spmd(nc, in_maps, core_ids=list(range(NCORES)),
                               trace=trace)
    return res, _host_combine(res.results)


def kernel(**inputs):
    _, (losses, precs) = run_cores(inputs, trace=False)
    return losses, precs


# revision 15
# speedup vs baseline: 1.2594x; 1.2594x over previous
"""Trainium2 Bass kernel for nn_LossComputation (multi-loss: instance CE,
global-align, CMPC CE, CMPM KL, max-hinge) on 8 NeuronCores.

Sharding: the classifier dim C (=11003, padded to 8*1376) of W/W2 is sharded
across the 8 cores for the four big [B,D]@[D,C] matmuls + CE partials; the
batch dim is sharded across cores for the BxB similarity losses. All
cross-core reductions are tiny [B]-vectors combined on the host.
"""
import sys

if "/opt/trn_rl_repo" not in sys.path:
    sys.path.insert(0, "/opt/trn_rl_repo")

import contextlib

import numpy as np

import concourse.bass as bass
import concourse.mybir as mybir
import concourse.tile as tile
from concourse.masks import make_identity
from concourse.vector_clock import ScopedClock, VectorClock
from concourse.bass_utils import run_bass_kernel_spmd

F32 = mybir.dt.float32
F32R = mybir.dt.float32r
AF = mybir.ActivationFunctionType
ALU = mybir.AluOpType
AX = mybir.AxisListType

SCALE = 28.0
MARGIN = 0.2
ALPHA, BETA = 0.6, 0.4
SCALE_POS, SCALE_NEG = 10.0, 40.0
EPS = 1e-8
LOG_EPS = float(np.log(np.float32(EPS)))

B, D, C = 1024, 1024, 11003
NCORES = 8
CS = 1376               # per-core padded shard width (8*1376 = 11008)
NPAD = NCORES * CS - C  # 5 pad columns, all on core 7
ROWS = B // NCORES      # 128 batch rows per core for the BxB losses
NT = 8                  # number of 128-row/128-col tiles in B or D
CHUNKS = [(0, 512), (512, 512), (1024, 352)]  # n-chunks of the C shard

_NC_CACHE = {}


# Tile emits one final drain carrying one sem-wait per logical proc; this
# walrus build rejects >2 sync waits on an SP CTRL instruction, so emit one
# drain per proc instead.
def _patched_drain_and_barrier(self, tick_clock, wait_clock):
    ticks = list(tick_clock.global_clock)
    n = len(ticks)
    for i in range(n):
        if ticks[i] == 0:
            continue
        sub = [0] * n
        sub[i] = ticks[i]
        d = self.nc.sync.drain()
        wait_clock.add_sem_waits(d.ins, ScopedClock({None: VectorClock(sub)}))
    self.nc.all_engine_barrier()
    assert self.sems is not None
    popped = self.nc._tile_sem_poison_stack.pop()
    assert popped is self._sem_poison
    self.nc.clear_and_free_semaphores(list(self.sems.allocated().values()))
    self.nc.all_engine_barrier()


def _split_excess_waits(nc, maxw=2):
    """Walrus in this build rejects instructions carrying more than `maxw`
    sync-wait commands. Split the excess onto fresh NoOps inserted right
    before the offending instruction on the same engine — the engine is
    in-order, so waiting across two adjacent instruction slots is
    equivalent to waiting on one."""
    n_split = 0
    for f in nc.m.functions:
        for b in f.blocks:
            lst = b.instructions
            i = 0
            while i < len(lst):
                ins = lst[i]
                si = ins.sync_info
                waits = list(si.on_wait) if si and si.on_wait else []
                if len(waits) > maxw:
                    keep = waits[-maxw:]
                    excess = waits[:-maxw]
                    si.on_wait = keep
                    pos = i
                    for j in range(0, len(excess), maxw):
                        chunk = excess[j : j + maxw]
                        nop = mybir.InstNoOp(
                            name=f"{ins.name}-wsplit{j}", ins=[], outs=[])
                        nop.engine = ins.engine
                        nop.sync_info = mybir.SyncInfo(on_wait=chunk,
                                                       on_update=[])
                        lst.insert(pos, nop)
                        pos += 1
                        n_split += 1
                    i = pos
                i += 1
    return n_split


def _apply_patches():
    tile.TileContext._drain_and_barrier = _patched_drain_and_barrier
    # The container's antenv package lacks axon_hooks; shim it so the
    # trace=True path (NTFF profiling) works instead of crashing, and skip
    # the cloud artifact upload which isn't available here.
    try:
        import types

        import antenv
        from concourse import bass_utils as _bu

        if "antenv.axon_hooks" not in sys.modules:
            hooks = types.ModuleType("antenv.axon_hooks")
            hooks._hook = None
            hooks.set_axon_ntff_profile_hook = lambda h: setattr(hooks, "_hook", h)
            hooks.get_axon_ntff_profile_hook = lambda: hooks._hook
            sys.modules["antenv.axon_hooks"] = hooks
            antenv.axon_hooks = hooks
            try:
                from trn_agent_boot.trn_boot import _ntff_profile_via_ctypes

                hooks._hook = _ntff_profile_via_ctypes("/opt/axon/libaxon_pjrt.so")
            except Exception:
                pass
        _bu.upload_artifacts = lambda tmpdir: "/tmp/bass_artifacts_noupload"
    except Exception:
        pass


def _bcast_row_ap(dram_t, width, parts=128):
    """AP reading a [1, width] DRAM row broadcast over `parts` partitions."""
    ap = dram_t.ap()
    return bass.AP(tensor=ap.tensor, offset=ap.offset, ap=[[0, parts], [1, width]])


def build_nc():
    nc = bass.Bass("TRN2", target_bir_lowering=False, debug=False,
                   num_devices=NCORES)

    # register activation bias constants (only 0.0/1.0 are pre-registered)
    for val in (-SCALE_NEG * BETA, SCALE_POS * ALPHA, MARGIN):
        ct = nc.alloc_sbuf_tensor(f"const-float32-{val}", [128, 1], F32)
        nc.gpsimd.memset(ct.ap(), val)
        nc.const_aps.aps[(F32, float(val))] = ct.ap()
    nc.all_engine_barrier()

    din = {}

    def dram_in(name, shape):
        din[name] = nc.dram_tensor(name, shape, F32, kind="ExternalInput")
        return din[name]

    dout = {}

    def dram_out(name, shape):
        dout[name] = nc.dram_tensor(name, shape, F32, kind="ExternalOutput")
        return dout[name]

    v_d = dram_in("v", [B, D])
    t_d = dram_in("t", [B, D])
    w_d = dram_in("w", [D, CS])
    w2_d = dram_in("w2", [D, CS])
    padmask_d = dram_in("padmask", [1, CS])
    onehot_d = dram_in("onehot", [B, 256])
    labrow_d = dram_in("labrow", [1, B])
    logmn_d = dram_in("logmn", [1, B])
    lab_local_d = dram_in("lab_local", [ROWS, 1])
    vloc_d = dram_in("vloc", [ROWS, D])
    tloc_d = dram_in("tloc", [ROWS, D])

    for nm in ("S_v", "S_t", "lab_v", "lab_t",
               "S2_v", "S2_t", "lab2_v", "lab2_t",
               "labE2_v", "labE2_t", "M2E_v", "M2E_t", "mh_im"):
        dram_out(nm, [B, 1])
    for nm in ("ga_n", "ga_m", "mh_s", "i2t", "t2i"):
        dram_out(nm, [ROWS, 1])

    v_tiled = v_d.ap().rearrange("(n p) d -> p n d", p=128)
    t_tiled = t_d.ap().rearrange("(n p) d -> p n d", p=128)
    w_tiled = w_d.ap().rearrange("(k p) c -> p k c", p=128)
    w2_tiled = w2_d.ap().rearrange("(k p) c -> p k c", p=128)

    with tile.TileContext(nc) as tc, contextlib.ExitStack() as ctx:
        persist = ctx.enter_context(tc.tile_pool(name="persist", bufs=1))
        ident = persist.tile([128, 128], F32)
        make_identity(nc, ident)
        ones_raw = persist.tile([128, 1], F32)
        nc.vector.memset(ones_raw, 1.0)
        ones_col = persist.tile([128, 1], F32)
        nc.vector.tensor_copy(out=ones_col.bitcast(F32R), in_=ones_raw)
        ones_row_raw = persist.tile([1, 128], F32)
        nc.vector.memset(ones_row_raw, 1.0)
        ones_row = persist.tile([1, 128], F32)
        nc.vector.tensor_copy(out=ones_row.bitcast(F32R), in_=ones_row_raw)

        def bcast_row(row_tile, out_tile, width):
            """[1, width] -> [128, width] via K=1 PE matmul + PSUM evict."""
            for lo in range(0, width, 512):
                nsz = min(512, width - lo)
                pb = psum_tr.tile([128, 512], F32, tag="tr")
                nc.tensor.matmul(pb[:, :nsz], lhsT=ones_row.bitcast(F32R),
                                 rhs=row_tile[:, lo : lo + nsz].bitcast(F32R),
                                 start=True, stop=True)
                nc.scalar.copy(out=out_tile[:, lo : lo + nsz], in_=pb[:, :nsz])

        vnT = persist.tile([128, NT, B], F32)   # vn^T as k-tiles of D
        tnT = persist.tile([128, NT, B], F32)
        rv_s = persist.tile([128, NT], F32)     # rowsum(v^2), col n = row-tile
        rt_s = persist.tile([128, NT], F32)
        riv_s = persist.tile([128, NT], F32)    # 1/||v_b||
        rit_s = persist.tile([128, NT], F32)
        d_s = persist.tile([128, NT], F32)      # diag(sim)
        p_s = persist.tile([128, NT], F32)      # v.tn = d*||v||
        q_s = persist.tile([128, NT], F32)      # t.vn = d*||t||
        scr8 = persist.tile([128, NT], F32)

        pool_w = ctx.enter_context(tc.tile_pool(name="pool_w", bufs=1))
        w_sb = pool_w.tile([128, NT, CS], F32, tag="w")

        pool_st = ctx.enter_context(tc.tile_pool(name="pool_st", bufs=2))

        psum_mm = ctx.enter_context(
            tc.tile_pool(name="psum_mm", bufs=4, space="PSUM"))
        psum_tr = ctx.enter_context(
            tc.tile_pool(name="psum_tr", bufs=2, space="PSUM"))
        psum_row = ctx.enter_context(
            tc.tile_pool(name="psum_row", bufs=2, space="PSUM"))

        # ---------------- Phase 0: norms + transposes ----------------
        with tc.tile_pool(name="pool_p0", bufs=1) as pool_p0:
            vfull = pool_p0.tile([128, NT, D], F32, tag="vf")
            tfull = pool_p0.tile([128, NT, D], F32, tag="tf")
            nc.sync.dma_start(out=vfull, in_=v_tiled)
            nc.sync.dma_start(out=tfull, in_=t_tiled)

            for n in range(NT):
                sq = pool_st.tile([128, D], F32, tag="p0sq")
                nc.scalar.activation(out=sq, in_=vfull[:, n, :], func=AF.Square,
                                     accum_out=rv_s[:, n : n + 1])
                sq2 = pool_st.tile([128, D], F32, tag="p0sq")
                nc.scalar.activation(out=sq2, in_=tfull[:, n, :], func=AF.Square,
                                     accum_out=rt_s[:, n : n + 1])
            nc.scalar.activation(out=scr8, in_=rv_s, func=AF.Sqrt)
            nc.vector.reciprocal(out=riv_s, in_=scr8)
            nc.scalar.activation(out=scr8, in_=rt_s, func=AF.Sqrt)
            nc.vector.reciprocal(out=rit_s, in_=scr8)

            for n in range(NT):
                nc.vector.tensor_scalar_mul(vfull[:, n, :], vfull[:, n, :],
                                            riv_s[:, n : n + 1])
                nc.vector.tensor_scalar_mul(tfull[:, n, :], tfull[:, n, :],
                                            rit_s[:, n : n + 1])
                dsc = pool_st.tile([128, D], F32, tag="p0sq")
                nc.vector.scalar_tensor_tensor(
                    out=dsc, in0=vfull[:, n, :], scalar=1.0,
                    in1=tfull[:, n, :], op0=ALU.mult, op1=ALU.mult,
                    accum_out=d_s[:, n : n + 1])

            # p = d*||v|| = d*rv*riv ; q = d*||t||
            nc.vector.tensor_tensor(out=scr8, in0=rv_s, in1=riv_s, op=ALU.mult)
            nc.vector.tensor_tensor(out=p_s, in0=d_s, in1=scr8, op=ALU.mult)
            nc.vector.tensor_tensor(out=scr8, in0=rt_s, in1=rit_s, op=ALU.mult)
            nc.vector.tensor_tensor(out=q_s, in0=d_s, in1=scr8, op=ALU.mult)

            # transposes: batch 4 row-tiles n per PSUM bank, evict 512 at once
            for src, dst in ((vfull, vnT), (tfull, tnT)):
                for m in range(NT):
                    for g in range(2):
                        ptile = psum_tr.tile([128, 512], F32, tag="tr")
                        for j in range(4):
                            n = 4 * g + j
                            nc.tensor.transpose(
                                out=ptile[:, 128 * j : 128 * (j + 1)],
                                in_=src[:, n, 128 * m : 128 * (m + 1)],
                                identity=ident)
                        nc.scalar.copy(
                            out=dst[:, m, 512 * g : 512 * (g + 1)].bitcast(F32R),
                            in_=ptile)

        # ---------------- main pool for phases A/C/B ----------------
        pool_ac = ctx.enter_context(tc.tile_pool(name="pool_ac", bufs=1))

        def col_scale_W(wt_ap, wsb, scale_const):
            """Stream W from DRAM: pass 1 colssq -> wsc row -> broadcast;
            pass 2 re-stream + scale into the fp32r-rounded wsb tile."""
            cs_row = pool_ac.tile([1, CS], F32, tag="rowA")
            for ci, (nlo, nsz) in enumerate(CHUNKS):
                pr = psum_row.tile([1, 512], F32, tag="vrow")
                for k in range(NT):
                    stg = pool_st.tile([128, 512], F32, tag="wsq")
                    nc.sync.dma_start(out=stg[:, :nsz],
                                      in_=wt_ap[:, k, nlo : nlo + nsz])
                    sq = pool_st.tile([128, 512], F32, tag="wsq")
                    nc.scalar.activation(out=sq[:, :nsz].bitcast(F32R),
                                         in_=stg[:, :nsz],
                                         func=AF.Square)
                    nc.tensor.matmul(pr[:, :nsz], lhsT=ones_col.bitcast(F32R),
                                     rhs=sq[:, :nsz].bitcast(F32R),
                                     start=(k == 0), stop=(k == NT - 1))
                nc.scalar.activation(out=cs_row[:, nlo : nlo + nsz],
                                     in_=pr[:, :nsz], func=AF.Sqrt)
            nc.vector.reciprocal(out=cs_row, in_=cs_row)
            nc.vector.tensor_scalar_mul(cs_row, cs_row, float(scale_const))
            pm = pool_ac.tile([1, CS], F32, tag="rowC")
            nc.sync.dma_start(out=pm, in_=padmask_d.ap())
            wsc_row = pool_ac.tile([1, CS], F32, tag="rowB")
            nc.vector.tensor_tensor(out=wsc_row.bitcast(F32R), in0=cs_row,
                                    in1=pm, op=ALU.mult)
            wsc_bc = pool_ac.tile([128, CS], F32, tag="wsc_bc")
            bcast_row(wsc_row, wsc_bc, CS)
            for k in range(NT):
                stg2 = pool_st.tile([128, CS], F32, tag="p0sq")
                nc.sync.dma_start(out=stg2, in_=wt_ap[:, k, :])
                nc.gpsimd.tensor_tensor(out=wsb[:, k, :].bitcast(F32R),
                                        in0=stg2,
                                        in1=wsc_bc, op=ALU.mult)

        def ce_branch(wsb, lhsT_sb, S_o, lab_o, labE_o=None, M2_o=None):
            """logits = lhsT^T @ wsb per m-tile; exp-sums + label logit
            (+ max-exp and label-exp when requested, for cmpc precision)."""
            for mt in range(NT):
                oh = pool_st.tile([128, 256], F32, tag="oh")
                nc.sync.dma_start(out=oh,
                                  in_=onehot_d.ap()[128 * mt : 128 * (mt + 1), :])
                sp = pool_st.tile([128, 4], F32, tag="sp")
                lab_col = pool_st.tile([128, 1], F32, tag="labc")
                if M2_o is not None:
                    mp = pool_st.tile([128, 4], F32, tag="mp")
                    labE_col = pool_st.tile([128, 1], F32, tag="labEc")
                for ci, (nlo, nsz) in enumerate(CHUNKS):
                    ps = psum_mm.tile([128, 512], F32, tag="mm")
                    for k in range(NT):
                        nc.tensor.matmul(
                            ps[:, :nsz],
                            lhsT=lhsT_sb[:, k, 128 * mt : 128 * (mt + 1)].bitcast(F32R),
                            rhs=wsb[:, k, nlo : nlo + nsz].bitcast(F32R),
                            start=(k == 0), stop=(k == NT - 1))
                    e = pool_st.tile([128, 512], F32, tag="e_scr")
                    nc.scalar.activation(out=e[:, :nsz], in_=ps[:, :nsz],
                                         func=AF.Exp,
                                         accum_out=sp[:, ci : ci + 1])
                    if ci == 0:
                        scr = pool_st.tile([128, 256], F32, tag="scr256")
                        nc.vector.scalar_tensor_tensor(
                            out=scr, in0=ps[:, 0:256], scalar=1.0, in1=oh,
                            op0=ALU.mult, op1=ALU.mult, accum_out=lab_col)
                        if labE_o is not None:
                            scr2 = pool_st.tile([128, 256], F32, tag="scr256")
                            nc.vector.scalar_tensor_tensor(
                                out=scr2, in0=e[:, 0:256], scalar=1.0, in1=oh,
                                op0=ALU.mult, op1=ALU.mult, accum_out=labE_col)
                    if M2_o is not None:
                        nc.vector.reduce_max(out=mp[:, ci : ci + 1],
                                             in_=e[:, :nsz], axis=AX.X)
                rs = slice(128 * mt, 128 * (mt + 1))
                s_col = pool_st.tile([128, 1], F32, tag="scol")
                nc.vector.reduce_sum(out=s_col, in_=sp[:, 0:3], axis=AX.X)
                nc.sync.dma_start(out=S_o.ap()[rs, :], in_=s_col)
                nc.sync.dma_start(out=lab_o.ap()[rs, :], in_=lab_col)
                if M2_o is not None:
                    m_col = pool_st.tile([128, 1], F32, tag="mcol")
                    nc.vector.reduce_max(out=m_col, in_=mp[:, 0:3], axis=AX.X)
                    nc.sync.dma_start(out=M2_o.ap()[rs, :], in_=m_col)
                    nc.sync.dma_start(out=labE_o.ap()[rs, :], in_=labE_col)

        # ---------------- Phase A: instance loss ----------------
        col_scale_W(w_tiled, w_sb, SCALE)
        ce_branch(w_sb, vnT, dout["S_v"], dout["lab_v"])
        ce_branch(w_sb, tnT, dout["S_t"], dout["lab_t"])

        # W2 reuses the same pool slot; its writer waits for W's last reader
        w2_sb = pool_w.tile([128, NT, CS], F32, tag="w")

        # ---------------- Phase C: BxB losses on the local row shard ----
        vloc = pool_ac.tile([128, D], F32, tag="cscA")
        tloc = pool_ac.tile([128, D], F32, tag="cscB")
        nc.sync.dma_start(out=vloc, in_=vloc_d.ap())
        nc.sync.dma_start(out=tloc, in_=tloc_d.ap())

        # local row stats: cols 0..7 = rvl, rtl, rivl, ritl, dloc, -, -, tmp
        vstat = pool_ac.tile([128, 8], F32, tag="vstat")
        sqv = pool_st.tile([128, D], F32, tag="p0sq")
        nc.scalar.activation(out=sqv, in_=vloc, func=AF.Square,
                             accum_out=vstat[:, 0:1])
        sqt = pool_st.tile([128, D], F32, tag="p0sq")
        nc.scalar.activation(out=sqt, in_=tloc, func=AF.Square,
                             accum_out=vstat[:, 1:2])
        nc.scalar.activation(out=vstat[:, 2:4], in_=vstat[:, 0:2], func=AF.Sqrt)
        nc.vector.reciprocal(out=vstat[:, 2:4], in_=vstat[:, 2:4])
        dsc = pool_st.tile([128, D], F32, tag="p0sq")
        nc.vector.scalar_tensor_tensor(
            out=dsc, in0=vloc, scalar=1.0, in1=tloc,
            op0=ALU.mult, op1=ALU.mult, accum_out=vstat[:, 4:5])
        nc.vector.tensor_tensor(out=vstat[:, 7:8], in0=vstat[:, 2:3],
                                in1=vstat[:, 3:4], op=ALU.mult)
        nc.vector.tensor_tensor(out=vstat[:, 4:5], in0=vstat[:, 4:5],
                                in1=vstat[:, 7:8], op=ALU.mult)

        vn_loc = pool_ac.tile([128, D], F32, tag="cscC")
        nc.vector.tensor_scalar_mul(vn_loc, vloc, vstat[:, 2:3])

        # transposes of vloc, tloc, vn_loc -> [128, k, 128] lhsT forms
        vT_loc = pool_ac.tile([128, NT, 128], F32, tag="vT_loc")
        tT_loc = pool_ac.tile([128, NT, 128], F32, tag="tT_loc")
        vnT_loc = pool_ac.tile([128, NT, 128], F32, tag="vnT_loc")
        for src, dst in ((vloc, vT_loc), (tloc, tT_loc), (vn_loc, vnT_loc)):
            for g in range(2):
                ptile = psum_tr.tile([128, 512], F32, tag="tr")
                for j in range(4):
                    k = 4 * g + j
                    nc.tensor.transpose(
                        out=ptile[:, 128 * j : 128 * (j + 1)],
                        in_=src[:, 128 * k : 128 * (k + 1)], identity=ident)
                for j in range(4):
                    nc.scalar.copy(out=dst[:, 4 * g + j, :].bitcast(F32R),
                                   in_=ptile[:, 128 * j : 128 * (j + 1)])

        # mask / notmask / sel2
        labrow_bc = pool_ac.tile([128, B], F32, tag="bc1")
        nc.sync.dma_start(out=labrow_bc, in_=_bcast_row_ap(labrow_d, B))
        lab_loc = pool_ac.tile([128, 1], F32, tag="lab_loc")
        nc.sync.dma_start(out=lab_loc, in_=lab_local_d.ap())
        mask = pool_ac.tile([128, B], F32, tag="mask")
        nc.vector.tensor_scalar(out=mask, in0=labrow_bc, scalar1=lab_loc,
                                scalar2=None, op0=ALU.is_equal)
        logmn_bc = pool_ac.tile([128, B], F32, tag="bc1")
        nc.sync.dma_start(out=logmn_bc, in_=_bcast_row_ap(logmn_d, B))
        sel2 = pool_ac.tile([128, B], F32, tag="sel2")
        nc.vector.tensor_scalar(out=sel2, in0=logmn_bc, scalar1=LOG_EPS,
                                scalar2=None, op0=ALU.subtract)
        nc.vector.tensor_tensor(out=sel2, in0=sel2, in1=mask, op=ALU.mult)
        nc.vector.tensor_scalar(out=sel2, in0=sel2, scalar1=LOG_EPS,
                                scalar2=None, op0=ALU.add)

        # d row (full diag) -> broadcast
        drow = pool_ac.tile([1, B], F32, tag="rowD")
        for n in range(NT):
            pr = psum_row.tile([1, 512], F32, tag="vrow")
            nc.tensor.transpose(out=pr[:, 0:128], in_=d_s[:, n : n + 1],
                                identity=ident)
            nc.scalar.copy(out=drow[:, 128 * n : 128 * (n + 1)].bitcast(F32R),
                           in_=pr[:, 0:128])
        drow_bc = pool_ac.tile([128, B], F32, tag="bc3")
        bcast_row(drow, drow_bc, B)

        # sim = vn_loc @ tn^T  [128, B]
        sim_sb = pool_ac.tile([128, B], F32, tag="sim")
        for n2 in range(2):
            ps = psum_mm.tile([128, 512], F32, tag="mm")
            for k in range(NT):
                nc.tensor.matmul(ps, lhsT=vnT_loc[:, k, :].bitcast(F32R),
                                 rhs=tnT[:, k, 512 * n2 : 512 * (n2 + 1)].bitcast(F32R),
                                 start=(k == 0), stop=(k == NT - 1))
            nc.scalar.copy(out=sim_sb[:, 512 * n2 : 512 * (n2 + 1)], in_=ps)

        # ---- global-align ----
        ga_n_col = pool_st.tile([128, 1], F32, tag="ga_n")
        ga_m_col = pool_st.tile([128, 1], F32, tag="ga_m")
        e_neg = pool_ac.tile([128, B], F32, tag="cscA")
        nc.scalar.activation(out=e_neg, in_=sim_sb, func=AF.Exp,
                             scale=SCALE_NEG, bias=-SCALE_NEG * BETA)
        ln_neg = pool_ac.tile([128, B], F32, tag="cscB")
        nc.scalar.activation(out=ln_neg, in_=e_neg, func=AF.Ln, bias=1.0,
                             accum_out=ga_n_col)
        e_pos = pool_ac.tile([128, B], F32, tag="cscA")
        nc.scalar.activation(out=e_pos, in_=sim_sb, func=AF.Exp,
                             scale=-SCALE_POS, bias=SCALE_POS * ALPHA)
        ln_pos = pool_ac.tile([128, B], F32, tag="cscC")
        nc.scalar.activation(out=ln_pos, in_=e_pos, func=AF.Ln, bias=1.0)
        nc.vector.tensor_tensor(out=ln_pos, in0=ln_pos, in1=ln_neg,
                                op=ALU.subtract)
        scr = pool_ac.tile([128, B], F32, tag="cscB")
        nc.vector.scalar_tensor_tensor(
            out=scr, in0=ln_pos, scalar=1.0, in1=mask,
            op0=ALU.mult, op1=ALU.mult, accum_out=ga_m_col)
        nc.sync.dma_start(out=dout["ga_n"].ap(), in_=ga_n_col)
        nc.sync.dma_start(out=dout["ga_m"].ap(), in_=ga_m_col)

        # ---- max-hinge ----
        u = pool_ac.tile([128, B], F32, tag="cscA")
        nc.vector.tensor_scalar(out=u, in0=sim_sb, scalar1=vstat[:, 4:5],
                                scalar2=None, op0=ALU.subtract)
        r = pool_ac.tile([128, B], F32, tag="cscB")
        nc.scalar.activation(out=r, in_=u, func=AF.Relu, bias=MARGIN)
        mh_s_col = pool_st.tile([128, 1], F32, tag="mh_s")
        scr = pool_ac.tile([128, B], F32, tag="cscA")
        nc.vector.tensor_tensor(out=scr, in0=r, in1=mask, op=ALU.mult)
        nc.vector.tensor_tensor(out=scr, in0=r, in1=scr, op=ALU.subtract)
        nc.vector.reduce_max(out=mh_s_col, in_=scr, axis=AX.X)
        nc.sync.dma_start(out=dout["mh_s"].ap(), in_=mh_s_col)

        u2 = pool_ac.tile([128, B], F32, tag="cscA")
        nc.vector.tensor_tensor(out=u2, in0=sim_sb, in1=drow_bc,
                                op=ALU.subtract)
        r2 = pool_ac.tile([128, B], F32, tag="cscB")
        nc.scalar.activation(out=r2, in_=u2, func=AF.Relu, bias=MARGIN)
        cim = pool_ac.tile([128, B], F32, tag="cscA")
        nc.vector.tensor_tensor(out=cim, in0=r2, in1=mask, op=ALU.mult)
        nc.vector.tensor_tensor(out=cim, in0=r2, in1=cim, op=ALU.subtract)
        for g in range(2):
            ptile = psum_tr.tile([128, 512], F32, tag="tr")
            for j in range(4):
                jt = 4 * g + j
                nc.tensor.transpose(out=ptile[:, 128 * j : 128 * (j + 1)],
                                    in_=cim[:, 128 * jt : 128 * (jt + 1)],
                                    identity=ident)
            for j in range(4):
                jt = 4 * g + j
                mh_col = pool_st.tile([128, 1], F32, tag="mhim")
                nc.vector.reduce_max(out=mh_col,
                                     in_=ptile[:, 128 * j : 128 * (j + 1)],
                                     axis=AX.X)
                nc.sync.dma_start(
                    out=dout["mh_im"].ap()[128 * jt : 128 * (jt + 1), :],
                    in_=mh_col)

        # ---- cmpm (i2t / t2i) ----
        def cmpm_branch(lhsT_loc, rhsT, out_name):
            Ecm = pool_ac.tile([128, B], F32, tag="cscA")
            t3 = pool_ac.tile([128, B], F32, tag="cscB")
            zpart = pool_st.tile([128, 2], F32, tag="zpart")
            for n2 in range(2):
                ps = psum_mm.tile([128, 512], F32, tag="mm")
                for k in range(NT):
                    nc.tensor.matmul(ps, lhsT=lhsT_loc[:, k, :].bitcast(F32R),
                                     rhs=rhsT[:, k, 512 * n2 : 512 * (n2 + 1)].bitcast(F32R),
                                     start=(k == 0), stop=(k == NT - 1))
                sl = slice(512 * n2, 512 * (n2 + 1))
                nc.scalar.activation(out=Ecm[:, sl], in_=ps, func=AF.Exp,
                                     accum_out=zpart[:, n2 : n2 + 1])
                nc.vector.tensor_tensor(out=t3[:, sl], in0=ps, in1=sel2[:, sl],
                                        op=ALU.subtract)
            Z = pool_st.tile([128, 1], F32, tag="Zc")
            nc.vector.reduce_sum(out=Z, in_=zpart, axis=AX.X)
            logZ = pool_st.tile([128, 1], F32, tag="logZc")
            nc.scalar.activation(out=logZ, in_=Z, func=AF.Ln)
            nc.vector.tensor_scalar(out=t3, in0=t3, scalar1=logZ,
                                    scalar2=None, op0=ALU.subtract)
            srow = pool_st.tile([128, 1], F32, tag="srow")
            scr = pool_ac.tile([128, B], F32, tag="cscC")
            nc.vector.scalar_tensor_tensor(
                out=scr, in0=t3, scalar=1.0, in1=Ecm,
                op0=ALU.mult, op1=ALU.mult, accum_out=srow)
            rz = pool_st.tile([128, 1], F32, tag="rzc")
            nc.vector.reciprocal(out=rz, in_=Z)
            ocol = pool_st.tile([128, 1], F32, tag="ocol")
            nc.vector.tensor_tensor(out=ocol, in0=srow, in1=rz, op=ALU.mult)
            nc.sync.dma_start(out=dout[out_name].ap(), in_=ocol)

        cmpm_branch(vT_loc, tnT, "i2t")
        cmpm_branch(tT_loc, vnT, "t2i")

        # ---------------- Phase B: cmpc ----------------
        col_scale_W(w2_tiled, w2_sb, 1.0)

        # p/q rows -> broadcast
        prow = pool_ac.tile([1, B], F32, tag="rowD")
        qrow = pool_ac.tile([1, B], F32, tag="rowE")
        for stat, row in ((p_s, prow), (q_s, qrow)):
            for n in range(NT):
                pr = psum_row.tile([1, 512], F32, tag="vrow")
                nc.tensor.transpose(out=pr[:, 0:128], in_=stat[:, n : n + 1],
                                    identity=ident)
                nc.scalar.copy(out=row[:, 128 * n : 128 * (n + 1)].bitcast(F32R),
                               in_=pr[:, 0:128])
        pbc = pool_ac.tile([128, B], F32, tag="bc1")
        bcast_row(prow, pbc, B)
        qbc = pool_ac.tile([128, B], F32, tag="bc3")
        bcast_row(qrow, qbc, B)

        # vptT = tnT * p (in place over tnT), then v-branch
        for k in range(NT):
            nc.vector.tensor_tensor(out=tnT[:, k, :].bitcast(F32R),
                                    in0=tnT[:, k, :], in1=pbc, op=ALU.mult)
        ce_branch(w2_sb, tnT, dout["S2_v"], dout["lab2_v"],
                  labE_o=dout["labE2_v"], M2_o=dout["M2E_v"])
        # tpvT = vnT * q (in place over vnT), then t-branch
        for k in range(NT):
            nc.vector.tensor_tensor(out=vnT[:, k, :].bitcast(F32R),
                                    in0=vnT[:, k, :], in1=qbc, op=ALU.mult)
        ce_branch(w2_sb, vnT, dout["S2_t"], dout["lab2_t"],
                  labE_o=dout["labE2_t"], M2_o=dout["M2E_t"])

    _split_excess_waits(nc, maxw=1)
    return nc


def _host_prep(inputs):
    v = np.ascontiguousarray(np.asarray(inputs["visual_embed"], np.float32))
    t = np.ascontiguousarray(np.asarray(inputs["textual_embed"], np.float32))
    W = np.asarray(inputs["W"], np.float32)
    W2 = np.asarray(inputs["W2"], np.float32)
    labels = np.asarray(inputs["labels"]).astype(np.int64)

    labf = labels.astype(np.float32)
    onehot = np.zeros((B, 256), np.float32)
    onehot[np.arange(B), labels] = 1.0
    counts = np.bincount(labels, minlength=256)[labels]
    norm = np.sqrt(counts.astype(np.float32))
    logmn = np.log((np.float32(1.0) / norm).astype(np.float32)
                   + np.float32(EPS)).astype(np.float32)

    pad = np.ones((D, NPAD), np.float32)
    in_maps = []
    for k in range(NCORES):
        lo = k * CS
        hi = min((k + 1) * CS, C)
        wk = W[:, lo:hi]
        w2k = W2[:, lo:hi]
        pmk = np.ones((1, CS), np.float32)
        if hi - lo < CS:
            wk = np.concatenate([wk, pad[:, : CS - (hi - lo)]], axis=1)
            w2k = np.concatenate([w2k, pad[:, : CS - (hi - lo)]], axis=1)
            pmk[0, hi - lo :] = 0.0
        rs = slice(k * ROWS, (k + 1) * ROWS)
        in_maps.append({
            "v": v, "t": t,
            "w": np.ascontiguousarray(wk),
            "w2": np.ascontiguousarray(w2k),
            "padmask": pmk,
            "onehot": onehot,
            "labrow": labf[None, :].copy(),
            "logmn": logmn[None, :].copy(),
            "lab_local": np.ascontiguousarray(labf[rs, None]),
            "vloc": np.ascontiguousarray(v[rs]),
            "tloc": np.ascontiguousarray(t[rs]),
        })
    return in_maps


def _host_combine(outs):
    f32 = np.float32

    def ce(Ss, labs):
        S = np.sum([o[Ss][:, 0] for o in outs], axis=0) - NPAD
        return np.log(S).mean() - outs[0][labs][:, 0].mean()

    instance = ce("S_v", "lab_v") + ce("S_t", "lab_t")
    cmpc = ce("S2_v", "lab2_v") + ce("S2_t", "lab2_t")

    M2g_v = np.max([o["M2E_v"][:, 0] for o in outs], axis=0)
    M2g_t = np.max([o["M2E_t"][:, 0] for o in outs], axis=0)
    prec_v = (outs[0]["labE2_v"][:, 0] == M2g_v).mean()
    prec_t = (outs[0]["labE2_t"][:, 0] == M2g_t).mean()

    ga = 2.0 * np.sum([o["ga_n"].sum() + o["ga_m"].sum() for o in outs]) / B
    cmpm = np.concatenate([o["i2t"][:, 0] for o in outs]).mean() + \
        np.concatenate([o["t2i"][:, 0] for o in outs]).mean()
    mh = np.sum([o["mh_s"].sum() for o in outs]) + \
        np.max([o["mh_im"][:, 0] for o in outs], axis=0).sum()

    losses = np.array([instance, ga, cmpc, cmpm, mh], f32)
    precs = np.array([prec_v, prec_t], f32)
    return losses, precs


def run_cores(inputs, trace=False):
    """Build+compile (cached), run on the 8 cores, return (res, combined)."""
    _apply_patches()
    if "nc" not in _NC_CACHE:
        _NC_CACHE["nc"] = build_nc()
    nc = _NC_CACHE["nc"]
    in_maps = _host_prep(inputs)
    res = run_bass_kernel_spmd(nc, in_maps, core_ids=list(range(NCORES)),
                               trace=trace)
    return res, _host_combine(res.results)


def kernel(**inputs):
    _, (losses, precs) = run_cores(inputs, trace=False)
    return losses, precs


# revision 19
# speedup vs baseline: 1.6149x; 1.2823x over previous
"""Trainium2 Bass kernel for nn_LossComputation (multi-loss: instance CE,
global-align, CMPC CE, CMPM KL, max-hinge) on 8 NeuronCores.

Sharding: the classifier dim C (=11003, padded to 8*1376) of W/W2 is sharded
across the 8 cores for the four big [B,D]@[D,C] matmuls + CE partials; the
batch dim is sharded across cores for the BxB similarity losses. All
cross-core reductions are tiny [B]-vectors combined on the host.
"""
import sys

if "/opt/trn_rl_repo" not in sys.path:
    sys.path.insert(0, "/opt/trn_rl_repo")

import contextlib

import numpy as np

import concourse.bass as bass
import concourse.mybir as mybir
import concourse.tile as tile
from concourse.masks import make_identity
from concourse.vector_clock import ScopedClock, VectorClock
from concourse.bass_utils import run_bass_kernel_spmd

F32 = mybir.dt.float32
F32R = mybir.dt.float32r
AF = mybir.ActivationFunctionType
ALU = mybir.AluOpType
AX = mybir.AxisListType

SCALE = 28.0
MARGIN = 0.2
ALPHA, BETA = 0.6, 0.4
SCALE_POS, SCALE_NEG = 10.0, 40.0
EPS = 1e-8
LOG_EPS = float(np.log(np.float32(EPS)))

B, D, C = 1024, 1024, 11003
NCORES = 8
CS = 1376               # per-core padded shard width (8*1376 = 11008)
NPAD = NCORES * CS - C  # 5 pad columns, all on core 7
ROWS = B // NCORES      # 128 batch rows per core for the BxB losses
NT = 8                  # number of 128-row/128-col tiles in B or D
CHUNKS = [(0, 512), (512, 512), (1024, 352)]  # n-chunks of the C shard

_NC_CACHE = {}


# Tile emits one final drain carrying one sem-wait per logical proc; this
# walrus build rejects >2 sync waits on an SP CTRL instruction, so emit one
# drain per proc instead.
def _patched_drain_and_barrier(self, tick_clock, wait_clock):
    ticks = list(tick_clock.global_clock)
    n = len(ticks)
    for i in range(n):
        if ticks[i] == 0:
            continue
        sub = [0] * n
        sub[i] = ticks[i]
        d = self.nc.sync.drain()
        wait_clock.add_sem_waits(d.ins, ScopedClock({None: VectorClock(sub)}))
    self.nc.all_engine_barrier()
    assert self.sems is not None
    popped = self.nc._tile_sem_poison_stack.pop()
    assert popped is self._sem_poison
    self.nc.clear_and_free_semaphores(list(self.sems.allocated().values()))
    self.nc.all_engine_barrier()


def _split_excess_waits(nc, maxw=2):
    """Walrus in this build rejects instructions carrying more than `maxw`
    sync-wait commands. Split the excess onto fresh NoOps inserted right
    before the offending instruction on the same engine — the engine is
    in-order, so waiting across two adjacent instruction slots is
    equivalent to waiting on one."""
    n_split = 0
    for f in nc.m.functions:
        for b in f.blocks:
            lst = b.instructions
            i = 0
            while i < len(lst):
                ins = lst[i]
                si = ins.sync_info
                waits = list(si.on_wait) if si and si.on_wait else []
                if len(waits) > maxw:
                    keep = waits[-maxw:]
                    excess = waits[:-maxw]
                    si.on_wait = keep
                    pos = i
                    for j in range(0, len(excess), maxw):
                        chunk = excess[j : j + maxw]
                        nop = mybir.InstNoOp(
                            name=f"{ins.name}-wsplit{j}", ins=[], outs=[])
                        nop.engine = ins.engine
                        nop.sync_info = mybir.SyncInfo(on_wait=chunk,
                                                       on_update=[])
                        lst.insert(pos, nop)
                        pos += 1
                        n_split += 1
                    i = pos
                i += 1
    return n_split


def _apply_patches():
    tile.TileContext._drain_and_barrier = _patched_drain_and_barrier
    # The container's antenv package lacks axon_hooks; shim it so the
    # trace=True path (NTFF profiling) works instead of crashing, and skip
    # the cloud artifact upload which isn't available here.
    try:
        import types

        import antenv
        from concourse import bass_utils as _bu

        if "antenv.axon_hooks" not in sys.modules:
            hooks = types.ModuleType("antenv.axon_hooks")
            hooks._hook = None
            hooks.set_axon_ntff_profile_hook = lambda h: setattr(hooks, "_hook", h)
            hooks.get_axon_ntff_profile_hook = lambda: hooks._hook
            sys.modules["antenv.axon_hooks"] = hooks
            antenv.axon_hooks = hooks
            try:
                from trn_agent_boot.trn_boot import _ntff_profile_via_ctypes

                hooks._hook = _ntff_profile_via_ctypes("/opt/axon/libaxon_pjrt.so")
            except Exception:
                pass
        _bu.upload_artifacts = lambda tmpdir: "/tmp/bass_artifacts_noupload"
    except Exception:
        pass


def _bcast_row_ap(dram_t, width, parts=128):
    """AP reading a [1, width] DRAM row broadcast over `parts` partitions."""
    ap = dram_t.ap()
    return bass.AP(tensor=ap.tensor, offset=ap.offset, ap=[[0, parts], [1, width]])


def build_nc():
    nc = bass.Bass("TRN2", target_bir_lowering=False, debug=False,
                   num_devices=NCORES)

    # register activation bias constants (only 0.0/1.0 are pre-registered)
    for val in (-SCALE_NEG * BETA, SCALE_POS * ALPHA, MARGIN):
        ct = nc.alloc_sbuf_tensor(f"const-float32-{val}", [128, 1], F32)
        nc.gpsimd.memset(ct.ap(), val)
        nc.const_aps.aps[(F32, float(val))] = ct.ap()
    nc.all_engine_barrier()

    din = {}

    def dram_in(name, shape):
        din[name] = nc.dram_tensor(name, shape, F32, kind="ExternalInput")
        return din[name]

    dout = {}

    def dram_out(name, shape):
        dout[name] = nc.dram_tensor(name, shape, F32, kind="ExternalOutput")
        return dout[name]

    v_d = dram_in("v", [128, NT * D])
    t_d = dram_in("t", [128, NT * D])
    w_d = dram_in("w", [128, NT * CS])
    w2_d = dram_in("w2", [128, NT * CS])
    padmask_d = dram_in("padmask", [1, CS])
    onehot_d = dram_in("onehot", [128, NT * 256])
    labrow_d = dram_in("labrow", [1, B])
    logmn_d = dram_in("logmn", [1, B])
    lab_local_d = dram_in("lab_local", [ROWS, 1])
    vloc_d = dram_in("vloc", [ROWS, D])
    tloc_d = dram_in("tloc", [ROWS, D])

    for nm in ("S_v", "S_t", "lab_v", "lab_t",
               "S2_v", "S2_t", "lab2_v", "lab2_t",
               "labE2_v", "labE2_t", "M2E_v", "M2E_t", "mh_im"):
        dram_out(nm, [B, 1])
    for nm in ("ga_n", "ga_m", "mh_s", "i2t", "t2i"):
        dram_out(nm, [ROWS, 1])

    v_tiled = v_d.ap().rearrange("p (n d) -> p n d", n=NT)
    t_tiled = t_d.ap().rearrange("p (n d) -> p n d", n=NT)
    w_tiled = w_d.ap().rearrange("p (k c) -> p k c", k=NT)
    w2_tiled = w2_d.ap().rearrange("p (k c) -> p k c", k=NT)
    onehot_tiled = onehot_d.ap().rearrange("p (n c) -> p n c", n=NT)

    with tile.TileContext(nc) as tc, contextlib.ExitStack() as ctx:
        persist = ctx.enter_context(tc.tile_pool(name="persist", bufs=1))
        ident = persist.tile([128, 128], F32)
        make_identity(nc, ident)
        ones_raw = persist.tile([128, 1], F32)
        nc.vector.memset(ones_raw, 1.0)
        ones_col = persist.tile([128, 1], F32)
        nc.vector.tensor_copy(out=ones_col.bitcast(F32R), in_=ones_raw)
        ones_row_raw = persist.tile([1, 128], F32)
        nc.vector.memset(ones_row_raw, 1.0)
        ones_row = persist.tile([1, 128], F32)
        nc.vector.tensor_copy(out=ones_row.bitcast(F32R), in_=ones_row_raw)

        def bcast_row(row_tile, out_tile, width):
            """[1, width] -> [128, width] via K=1 PE matmul + PSUM evict."""
            for lo in range(0, width, 512):
                nsz = min(512, width - lo)
                pb = psum_tr.tile([128, 512], F32, tag="tr")
                nc.tensor.matmul(pb[:, :nsz], lhsT=ones_row.bitcast(F32R),
                                 rhs=row_tile[:, lo : lo + nsz].bitcast(F32R),
                                 start=True, stop=True)
                nc.scalar.copy(out=out_tile[:, lo : lo + nsz], in_=pb[:, :nsz])

        vnT = persist.tile([128, NT, B], F32)   # vn^T as k-tiles of D
        tnT = persist.tile([128, NT, B], F32)
        rv_s = persist.tile([128, NT], F32)     # rowsum(v^2), col n = row-tile
        rt_s = persist.tile([128, NT], F32)
        riv_s = persist.tile([128, NT], F32)    # 1/||v_b||
        rit_s = persist.tile([128, NT], F32)
        d_s = persist.tile([128, NT], F32)      # diag(sim)
        p_s = persist.tile([128, NT], F32)      # v.tn = d*||v||
        q_s = persist.tile([128, NT], F32)      # t.vn = d*||t||
        scr8 = persist.tile([128, NT], F32)

        pool_w = ctx.enter_context(tc.tile_pool(name="pool_w", bufs=1))
        w_sb = pool_w.tile([128, NT, CS], F32, tag="w")

        pool_st = ctx.enter_context(tc.tile_pool(name="pool_st", bufs=2))

        psum_mm = ctx.enter_context(
            tc.tile_pool(name="psum_mm", bufs=3, space="PSUM"))
        psum_tr = ctx.enter_context(
            tc.tile_pool(name="psum_tr", bufs=2, space="PSUM"))
        psum_row = ctx.enter_context(
            tc.tile_pool(name="psum_row", bufs=3, space="PSUM"))

        # ---------------- Phase 0: norms + transposes (two halves) ------
        with tc.tile_pool(name="pool_p0", bufs=1) as pool_p0:
            for h in range(2):
                hs = slice(4 * h, 4 * h + 4)
                th = pool_p0.tile([128, 4, D], F32, tag="th")
                vh = pool_p0.tile([128, 4, D], F32, tag="vh")
                nc.sync.dma_start(out=th, in_=t_tiled[:, hs, :])
                nc.sync.dma_start(out=vh, in_=v_tiled[:, hs, :])
                for j in range(4):
                    n = 4 * h + j
                    sq = pool_st.tile([128, D], F32, tag="p0sq")
                    nc.scalar.activation(out=sq, in_=vh[:, j, :],
                                         func=AF.Square,
                                         accum_out=rv_s[:, n : n + 1])
                    sq2 = pool_st.tile([128, D], F32, tag="p0sq")
                    nc.scalar.activation(out=sq2, in_=th[:, j, :],
                                         func=AF.Square,
                                         accum_out=rt_s[:, n : n + 1])
                nc.scalar.activation(out=scr8[:, hs], in_=rv_s[:, hs],
                                     func=AF.Sqrt)
                nc.vector.reciprocal(out=riv_s[:, hs], in_=scr8[:, hs])
                nc.scalar.activation(out=scr8[:, hs], in_=rt_s[:, hs],
                                     func=AF.Sqrt)
                nc.vector.reciprocal(out=rit_s[:, hs], in_=scr8[:, hs])

                for j in range(4):
                    n = 4 * h + j
                    nc.vector.tensor_scalar_mul(vh[:, j, :], vh[:, j, :],
                                                riv_s[:, n : n + 1])
                    nc.vector.tensor_scalar_mul(th[:, j, :], th[:, j, :],
                                                rit_s[:, n : n + 1])
                    dsc = pool_st.tile([128, D], F32, tag="p0sq")
                    nc.vector.scalar_tensor_tensor(
                        out=dsc, in0=vh[:, j, :], scalar=1.0,
                        in1=th[:, j, :], op0=ALU.mult, op1=ALU.mult,
                        accum_out=d_s[:, n : n + 1])

                for srch, dst in ((vh, vnT), (th, tnT)):
                    for m in range(NT):
                        ptile = psum_tr.tile([128, 512], F32, tag="tr")
                        for j in range(4):
                            nc.tensor.transpose(
                                out=ptile[:, 128 * j : 128 * (j + 1)],
                                in_=srch[:, j, 128 * m : 128 * (m + 1)],
                                identity=ident)
                        nc.scalar.copy(
                            out=dst[:, m, 512 * h : 512 * (h + 1)].bitcast(F32R),
                            in_=ptile)

            # p = d*||v|| = d*rv*riv ; q = d*||t||
            nc.vector.tensor_tensor(out=scr8, in0=rv_s, in1=riv_s, op=ALU.mult)
            nc.vector.tensor_tensor(out=p_s, in0=d_s, in1=scr8, op=ALU.mult)
            nc.vector.tensor_tensor(out=scr8, in0=rt_s, in1=rit_s, op=ALU.mult)
            nc.vector.tensor_tensor(out=q_s, in0=d_s, in1=scr8, op=ALU.mult)

        # ---------------- main pool for phases A/C/B ----------------
        pool_ac = ctx.enter_context(tc.tile_pool(name="pool_ac", bufs=1))

        def col_pass1(wt_ap, scale_const, row_tag):
            """Stream W k-tiles once: colssq -> wsc row [1, CS] (rounded)."""
            cs_row = persist.tile([1, CS], F32, tag="rowA")
            pr0 = psum_row.tile([1, 512], F32, tag="vrow")
            pr1 = psum_row.tile([1, 512], F32, tag="vrow")
            pr2 = psum_row.tile([1, 512], F32, tag="vrow")
            prs = [pr0, pr1, pr2]
            for k in range(NT):
                stg = pool_st.tile([128, CS], F32, tag="wstg")
                nc.sync.dma_start(out=stg, in_=wt_ap[:, k, :])
                for ci, (nlo, nsz) in enumerate(CHUNKS):
                    sq = pool_st.tile([128, 512], F32, tag="wsq")
                    nc.scalar.activation(out=sq[:, :nsz].bitcast(F32R),
                                         in_=stg[:, nlo : nlo + nsz],
                                         func=AF.Square)
                    nc.tensor.matmul(prs[ci][:, :nsz],
                                     lhsT=ones_col.bitcast(F32R),
                                     rhs=sq[:, :nsz].bitcast(F32R),
                                     start=(k == 0), stop=(k == NT - 1))
            for ci, (nlo, nsz) in enumerate(CHUNKS):
                nc.scalar.activation(out=cs_row[:, nlo : nlo + nsz],
                                     in_=prs[ci][:, :nsz], func=AF.Sqrt)
            nc.vector.reciprocal(out=cs_row, in_=cs_row)
            nc.vector.tensor_scalar_mul(cs_row, cs_row, float(scale_const))
            pm = persist.tile([1, CS], F32, tag="rowC")
            nc.sync.dma_start(out=pm, in_=padmask_d.ap())
            wsc_row = persist.tile([1, CS], F32, tag=row_tag)
            nc.vector.tensor_tensor(out=wsc_row.bitcast(F32R), in0=cs_row,
                                    in1=pm, op=ALU.mult)
            return wsc_row

        def col_pass2(wt_ap, wsb, wsc_row):
            """Re-stream W, scale columns into the fp32r-rounded wsb tile."""
            wsc_bc = pool_ac.tile([128, CS], F32, tag="wsc_bc")
            bcast_row(wsc_row, wsc_bc, CS)
            for k in range(NT):
                stg2 = pool_st.tile([128, CS], F32, tag="wstg")
                nc.sync.dma_start(out=stg2, in_=wt_ap[:, k, :])
                nc.gpsimd.tensor_tensor(out=wsb[:, k, :].bitcast(F32R),
                                        in0=stg2,
                                        in1=wsc_bc, op=ALU.mult)

        wsc_row_w = col_pass1(w_tiled, SCALE, "wscrow")

        def ce_branch(wsb, lhsT_sb, S_o, lab_o, labE_o=None, M2_o=None):
            """logits = lhsT^T @ wsb per m-tile; exp-sums + label logit
            (+ max-exp and label-exp when requested, for cmpc precision)."""
            for mt in range(NT):
                oh = pool_st.tile([128, 256], F32, tag="oh")
                nc.sync.dma_start(out=oh, in_=onehot_tiled[:, mt, :])
                sp = pool_st.tile([128, 4], F32, tag="sp")
                lab_col = pool_st.tile([128, 1], F32, tag="labc")
                if M2_o is not None:
                    mp = pool_st.tile([128, 4], F32, tag="mp")
                    labE_col = pool_st.tile([128, 1], F32, tag="labEc")
                for ci, (nlo, nsz) in enumerate(CHUNKS):
                    ps = psum_mm.tile([128, 512], F32, tag="mm")
                    for k in range(NT):
                        nc.tensor.matmul(
                            ps[:, :nsz],
                            lhsT=lhsT_sb[:, k, 128 * mt : 128 * (mt + 1)].bitcast(F32R),
                            rhs=wsb[:, k, nlo : nlo + nsz].bitcast(F32R),
                            start=(k == 0), stop=(k == NT - 1))
                    e = pool_st.tile([128, 512], F32, tag="e_scr")
                    nc.scalar.activation(out=e[:, :nsz], in_=ps[:, :nsz],
                                         func=AF.Exp,
                                         accum_out=sp[:, ci : ci + 1])
                    if ci == 0:
                        scr = pool_st.tile([128, 256], F32, tag="scr256")
                        nc.vector.scalar_tensor_tensor(
                            out=scr, in0=ps[:, 0:256], scalar=1.0, in1=oh,
                            op0=ALU.mult, op1=ALU.mult, accum_out=lab_col)
                        if labE_o is not None:
                            scr2 = pool_st.tile([128, 256], F32, tag="scr256")
                            nc.vector.scalar_tensor_tensor(
                                out=scr2, in0=e[:, 0:256], scalar=1.0, in1=oh,
                                op0=ALU.mult, op1=ALU.mult, accum_out=labE_col)
                    if M2_o is not None:
                        nc.vector.reduce_max(out=mp[:, ci : ci + 1],
                                             in_=e[:, :nsz], axis=AX.X)
                rs = slice(128 * mt, 128 * (mt + 1))
                s_col = pool_st.tile([128, 1], F32, tag="scol")
                nc.vector.reduce_sum(out=s_col, in_=sp[:, 0:3], axis=AX.X)
                nc.sync.dma_start(out=S_o.ap()[rs, :], in_=s_col)
                nc.sync.dma_start(out=lab_o.ap()[rs, :], in_=lab_col)
                if M2_o is not None:
                    m_col = pool_st.tile([128, 1], F32, tag="mcol")
                    nc.vector.reduce_max(out=m_col, in_=mp[:, 0:3], axis=AX.X)
                    nc.sync.dma_start(out=M2_o.ap()[rs, :], in_=m_col)
                    nc.sync.dma_start(out=labE_o.ap()[rs, :], in_=labE_col)

        # ---------------- Phase A: instance loss ----------------
        col_pass2(w_tiled, w_sb, wsc_row_w)
        ce_branch(w_sb, vnT, dout["S_v"], dout["lab_v"])
        ce_branch(w_sb, tnT, dout["S_t"], dout["lab_t"])

        # W2 colssq streams during phase A/C; same wsc row slot (WAR-ordered)
        wsc_row_w2 = col_pass1(w2_tiled, 1.0, "wscrow")
        # W2 reuses the same pool slot; its writer waits for W's last reader
        w2_sb = pool_w.tile([128, NT, CS], F32, tag="w")

        # ---------------- Phase C: BxB losses on the local row shard ----
        vloc = pool_ac.tile([128, D], F32, tag="cscA")
        tloc = pool_ac.tile([128, D], F32, tag="cscB")
        nc.sync.dma_start(out=vloc, in_=vloc_d.ap())
        nc.sync.dma_start(out=tloc, in_=tloc_d.ap())

        # local row stats: cols 0..7 = rvl, rtl, rivl, ritl, dloc, -, -, tmp
        vstat = pool_ac.tile([128, 8], F32, tag="vstat")
        sqv = pool_st.tile([128, D], F32, tag="p0sq")
        nc.scalar.activation(out=sqv, in_=vloc, func=AF.Square,
                             accum_out=vstat[:, 0:1])
        sqt = pool_st.tile([128, D], F32, tag="p0sq")
        nc.scalar.activation(out=sqt, in_=tloc, func=AF.Square,
                             accum_out=vstat[:, 1:2])
        nc.scalar.activation(out=vstat[:, 2:4], in_=vstat[:, 0:2], func=AF.Sqrt)
        nc.vector.reciprocal(out=vstat[:, 2:4], in_=vstat[:, 2:4])
        dsc = pool_st.tile([128, D], F32, tag="p0sq")
        nc.vector.scalar_tensor_tensor(
            out=dsc, in0=vloc, scalar=1.0, in1=tloc,
            op0=ALU.mult, op1=ALU.mult, accum_out=vstat[:, 4:5])
        nc.vector.tensor_tensor(out=vstat[:, 7:8], in0=vstat[:, 2:3],
                                in1=vstat[:, 3:4], op=ALU.mult)
        nc.vector.tensor_tensor(out=vstat[:, 4:5], in0=vstat[:, 4:5],
                                in1=vstat[:, 7:8], op=ALU.mult)

        # transposes of vloc, tloc -> [128, k, 128] lhsT forms
        vT_loc = pool_ac.tile([128, NT, 128], F32, tag="vT_loc")
        tT_loc = pool_ac.tile([128, NT, 128], F32, tag="tT_loc")
        for src, dst in ((vloc, vT_loc), (tloc, tT_loc)):
            for g in range(2):
                ptile = psum_tr.tile([128, 512], F32, tag="tr")
                for j in range(4):
                    k = 4 * g + j
                    nc.tensor.transpose(
                        out=ptile[:, 128 * j : 128 * (j + 1)],
                        in_=src[:, 128 * k : 128 * (k + 1)], identity=ident)
                for j in range(4):
                    nc.scalar.copy(out=dst[:, 4 * g + j, :].bitcast(F32R),
                                   in_=ptile[:, 128 * j : 128 * (j + 1)])

        # mask / notmask / sel2
        labrow_bc = pool_ac.tile([128, B], F32, tag="bc1")
        nc.sync.dma_start(out=labrow_bc, in_=_bcast_row_ap(labrow_d, B))
        lab_loc = pool_ac.tile([128, 1], F32, tag="lab_loc")
        nc.sync.dma_start(out=lab_loc, in_=lab_local_d.ap())
        mask = pool_ac.tile([128, B], F32, tag="mask")
        nc.vector.tensor_scalar(out=mask, in0=labrow_bc, scalar1=lab_loc,
                                scalar2=None, op0=ALU.is_equal)
        sel2 = pool_ac.tile([128, B], F32, tag="bc1")
        nc.sync.dma_start(out=sel2, in_=_bcast_row_ap(logmn_d, B))
        nc.vector.tensor_scalar(out=sel2, in0=sel2, scalar1=LOG_EPS,
                                scalar2=None, op0=ALU.subtract)
        nc.vector.tensor_tensor(out=sel2, in0=sel2, in1=mask, op=ALU.mult)
        nc.vector.tensor_scalar(out=sel2, in0=sel2, scalar1=LOG_EPS,
                                scalar2=None, op0=ALU.add)

        # d row (full diag) -> broadcast
        drow = pool_ac.tile([1, B], F32, tag="rowD")
        for n in range(NT):
            pr = psum_row.tile([1, 512], F32, tag="vrow")
            nc.tensor.transpose(out=pr[:, 0:128], in_=d_s[:, n : n + 1],
                                identity=ident)
            nc.scalar.copy(out=drow[:, 128 * n : 128 * (n + 1)].bitcast(F32R),
                           in_=pr[:, 0:128])
        drow_bc = pool_ac.tile([128, B], F32, tag="bc3")
        bcast_row(drow, drow_bc, B)

        # sim = diag(1/||v||) * (vloc @ tn^T)  [128, B]: row scale on evict
        sim_sb = pool_ac.tile([128, B], F32, tag="sim")
        for n2 in range(2):
            ps = psum_mm.tile([128, 512], F32, tag="mm")
            for k in range(NT):
                nc.tensor.matmul(ps, lhsT=vT_loc[:, k, :].bitcast(F32R),
                                 rhs=tnT[:, k, 512 * n2 : 512 * (n2 + 1)].bitcast(F32R),
                                 start=(k == 0), stop=(k == NT - 1))
            nc.vector.tensor_scalar(out=sim_sb[:, 512 * n2 : 512 * (n2 + 1)],
                                    in0=ps, scalar1=vstat[:, 2:3],
                                    scalar2=None, op0=ALU.mult)

        # ---- global-align ----
        ga_n_col = pool_st.tile([128, 1], F32, tag="ga_n")
        ga_m_col = pool_st.tile([128, 1], F32, tag="ga_m")
        e_neg = pool_ac.tile([128, B], F32, tag="cscA")
        nc.scalar.activation(out=e_neg, in_=sim_sb, func=AF.Exp,
                             scale=SCALE_NEG, bias=-SCALE_NEG * BETA)
        ln_neg = pool_ac.tile([128, B], F32, tag="cscB")
        nc.scalar.activation(out=ln_neg, in_=e_neg, func=AF.Ln, bias=1.0,
                             accum_out=ga_n_col)
        e_pos = pool_ac.tile([128, B], F32, tag="cscA")
        nc.scalar.activation(out=e_pos, in_=sim_sb, func=AF.Exp,
                             scale=-SCALE_POS, bias=SCALE_POS * ALPHA)
        nc.scalar.activation(out=e_pos, in_=e_pos, func=AF.Ln, bias=1.0)
        nc.vector.tensor_tensor(out=e_pos, in0=e_pos, in1=ln_neg,
                                op=ALU.subtract)
        scr = pool_ac.tile([128, B], F32, tag="cscB")
        nc.vector.scalar_tensor_tensor(
            out=scr, in0=e_pos, scalar=1.0, in1=mask,
            op0=ALU.mult, op1=ALU.mult, accum_out=ga_m_col)
        nc.sync.dma_start(out=dout["ga_n"].ap(), in_=ga_n_col)
        nc.sync.dma_start(out=dout["ga_m"].ap(), in_=ga_m_col)

        # ---- max-hinge ----
        u = pool_ac.tile([128, B], F32, tag="cscA")
        nc.vector.tensor_scalar(out=u, in0=sim_sb, scalar1=vstat[:, 4:5],
                                scalar2=None, op0=ALU.subtract)
        r = pool_ac.tile([128, B], F32, tag="cscB")
        nc.scalar.activation(out=r, in_=u, func=AF.Relu, bias=MARGIN)
        mh_s_col = pool_st.tile([128, 1], F32, tag="mh_s")
        scr = pool_ac.tile([128, B], F32, tag="cscA")
        nc.vector.tensor_tensor(out=scr, in0=r, in1=mask, op=ALU.mult)
        nc.vector.tensor_tensor(out=scr, in0=r, in1=scr, op=ALU.subtract)
        nc.vector.reduce_max(out=mh_s_col, in_=scr, axis=AX.X)
        nc.sync.dma_start(out=dout["mh_s"].ap(), in_=mh_s_col)

        u2 = pool_ac.tile([128, B], F32, tag="cscA")
        nc.vector.tensor_tensor(out=u2, in0=sim_sb, in1=drow_bc,
                                op=ALU.subtract)
        r2 = pool_ac.tile([128, B], F32, tag="cscB")
        nc.scalar.activation(out=r2, in_=u2, func=AF.Relu, bias=MARGIN)
        cim = pool_ac.tile([128, B], F32, tag="cscA")
        nc.vector.tensor_tensor(out=cim, in0=r2, in1=mask, op=ALU.mult)
        nc.vector.tensor_tensor(out=cim, in0=r2, in1=cim, op=ALU.subtract)
        for g in range(2):
            ptile = psum_tr.tile([128, 512], F32, tag="tr")
            for j in range(4):
                jt = 4 * g + j
                nc.tensor.transpose(out=ptile[:, 128 * j : 128 * (j + 1)],
                                    in_=cim[:, 128 * jt : 128 * (jt + 1)],
                                    identity=ident)
            for j in range(4):
                jt = 4 * g + j
                mh_col = pool_st.tile([128, 1], F32, tag="mhim")
                nc.vector.reduce_max(out=mh_col,
                                     in_=ptile[:, 128 * j : 128 * (j + 1)],
                                     axis=AX.X)
                nc.sync.dma_start(
                    out=dout["mh_im"].ap()[128 * jt : 128 * (jt + 1), :],
                    in_=mh_col)

        # ---- cmpm (i2t / t2i) ----
        def cmpm_branch(lhsT_loc, rhsT, out_name):
            Ecm = pool_ac.tile([128, B], F32, tag="cscA")
            t3 = pool_ac.tile([128, B], F32, tag="cscB")
            zpart = pool_st.tile([128, 2], F32, tag="zpart")
            for n2 in range(2):
                ps = psum_mm.tile([128, 512], F32, tag="mm")
                for k in range(NT):
                    nc.tensor.matmul(ps, lhsT=lhsT_loc[:, k, :].bitcast(F32R),
                                     rhs=rhsT[:, k, 512 * n2 : 512 * (n2 + 1)].bitcast(F32R),
                                     start=(k == 0), stop=(k == NT - 1))
                sl = slice(512 * n2, 512 * (n2 + 1))
                nc.scalar.activation(out=Ecm[:, sl], in_=ps, func=AF.Exp,
                                     accum_out=zpart[:, n2 : n2 + 1])
                nc.vector.tensor_tensor(out=t3[:, sl], in0=ps, in1=sel2[:, sl],
                                        op=ALU.subtract)
            Z = pool_st.tile([128, 1], F32, tag="Zc")
            nc.vector.reduce_sum(out=Z, in_=zpart, axis=AX.X)
            logZ = pool_st.tile([128, 1], F32, tag="logZc")
            nc.scalar.activation(out=logZ, in_=Z, func=AF.Ln)
            nc.vector.tensor_scalar(out=t3, in0=t3, scalar1=logZ,
                                    scalar2=None, op0=ALU.subtract)
            srow = pool_st.tile([128, 1], F32, tag="srow")
            nc.vector.scalar_tensor_tensor(
                out=t3, in0=t3, scalar=1.0, in1=Ecm,
                op0=ALU.mult, op1=ALU.mult, accum_out=srow)
            rz = pool_st.tile([128, 1], F32, tag="rzc")
            nc.vector.reciprocal(out=rz, in_=Z)
            ocol = pool_st.tile([128, 1], F32, tag="ocol")
            nc.vector.tensor_tensor(out=ocol, in0=srow, in1=rz, op=ALU.mult)
            nc.sync.dma_start(out=dout[out_name].ap(), in_=ocol)

        cmpm_branch(vT_loc, tnT, "i2t")
        cmpm_branch(tT_loc, vnT, "t2i")

        # ---------------- Phase B: cmpc ----------------
        col_pass2(w2_tiled, w2_sb, wsc_row_w2)

        # p/q rows -> broadcast
        prow = pool_ac.tile([1, B], F32, tag="rowD")
        qrow = pool_ac.tile([1, B], F32, tag="rowE")
        for stat, row in ((p_s, prow), (q_s, qrow)):
            for n in range(NT):
                pr = psum_row.tile([1, 512], F32, tag="vrow")
                nc.tensor.transpose(out=pr[:, 0:128], in_=stat[:, n : n + 1],
                                    identity=ident)
                nc.scalar.copy(out=row[:, 128 * n : 128 * (n + 1)].bitcast(F32R),
                               in_=pr[:, 0:128])
        pbc = pool_ac.tile([128, B], F32, tag="bc1")
        bcast_row(prow, pbc, B)
        qbc = pool_ac.tile([128, B], F32, tag="bc3")
        bcast_row(qrow, qbc, B)

        # vptT = tnT * p (in place over tnT), then v-branch
        for k in range(NT):
            nc.vector.tensor_tensor(out=tnT[:, k, :].bitcast(F32R),
                                    in0=tnT[:, k, :], in1=pbc, op=ALU.mult)
        ce_branch(w2_sb, tnT, dout["S2_v"], dout["lab2_v"],
                  labE_o=dout["labE2_v"], M2_o=dout["M2E_v"])
        # tpvT = vnT * q (in place over vnT), then t-branch
        for k in range(NT):
            nc.vector.tensor_tensor(out=vnT[:, k, :].bitcast(F32R),
                                    in0=vnT[:, k, :], in1=qbc, op=ALU.mult)
        ce_branch(w2_sb, vnT, dout["S2_t"], dout["lab2_t"],
                  labE_o=dout["labE2_t"], M2_o=dout["M2E_t"])

    _split_excess_waits(nc, maxw=1)
    return nc


def _host_prep(inputs):
    v = np.ascontiguousarray(np.asarray(inputs["visual_embed"], np.float32))
    t = np.ascontiguousarray(np.asarray(inputs["textual_embed"], np.float32))
    W = np.asarray(inputs["W"], np.float32)
    W2 = np.asarray(inputs["W2"], np.float32)
    labels = np.asarray(inputs["labels"]).astype(np.int64)

    labf = labels.astype(np.float32)
    onehot = np.zeros((B, 256), np.float32)
    onehot[np.arange(B), labels] = 1.0
    counts = np.bincount(labels, minlength=256)[labels]
    norm = np.sqrt(counts.astype(np.float32))
    logmn = np.log((np.float32(1.0) / norm).astype(np.float32)
                   + np.float32(EPS)).astype(np.float32)

    pad = np.ones((D, NPAD), np.float32)
    in_maps = []
    for k in range(NCORES):
        lo = k * CS
        hi = min((k + 1) * CS, C)
        wk = W[:, lo:hi]
        w2k = W2[:, lo:hi]
        pmk = np.ones((1, CS), np.float32)
        if hi - lo < CS:
            wk = np.concatenate([wk, pad[:, : CS - (hi - lo)]], axis=1)
            w2k = np.concatenate([w2k, pad[:, : CS - (hi - lo)]], axis=1)
            pmk[0, hi - lo :] = 0.0
        rs = slice(k * ROWS, (k + 1) * ROWS)
        in_maps.append({
            "v": v, "t": t,
            "w": np.ascontiguousarray(wk),
            "w2": np.ascontiguousarray(w2k),
            "padmask": pmk,
            "onehot": onehot,
            "labrow": labf[None, :].copy(),
            "logmn": logmn[None, :].copy(),
            "lab_local": np.ascontiguousarray(labf[rs, None]),
            "vloc": np.ascontiguousarray(v[rs]),
            "tloc": np.ascontiguousarray(t[rs]),
        })
    return in_maps


def _host_combine(outs):
    f32 = np.float32

    def ce(Ss, labs):
        S = np.sum([o[Ss][:, 0] for o in outs], axis=0) - NPAD
        return np.log(S).mean() - outs[0][labs][:, 0].mean()

    instance = ce("S_v", "lab_v") + ce("S_t", "lab_t")
    cmpc = ce("S2_v", "lab2_v") + ce("S2_t", "lab2_t")

    M2g_v = np.max([o["M2E_v"][:, 0] for o in outs], axis=0)
    M2g_t = np.max([o["M2E_t"][:, 0] for o in outs], axis=0)
    prec_v = (outs[0]["labE2_v"][:, 0] == M2g_v).mean()
    prec_t = (outs[0]["labE2_t"][:, 0] == M2g_t).mean()

    ga = 2.0 * np.sum([o["ga_n"].sum() + o["ga_m"].sum() for o in outs]) / B
    cmpm = np.concatenate([o["i2t"][:, 0] for o in outs]).mean() + \
        np.concatenate([o["t2i"][:, 0] for o in outs]).mean()
    mh = np.sum([o["mh_s"].sum() for o in outs]) + \
        np.max([o["mh_im"][:, 0] for o in outs], axis=0).sum()

    losses = np.array([instance, ga, cmpc, cmpm, mh], f32)
    precs = np.array([prec_v, prec_t], f32)
    return losses, precs


def run_cores(inputs, trace=False):
    """Build+compile (cached), run on the 8 cores, return (res, combined)."""
    _apply_patches()
    if "nc" not in _NC_CACHE:
        _NC_CACHE["nc"] = build_nc()
    nc = _NC_CACHE["nc"]
    in_maps = _host_prep(inputs)
    res = run_bass_kernel_spmd(nc, in_maps, core_ids=list(range(NCORES)),
                               trace=trace)
    return res, _host_combine(res.results)


def kernel(**inputs):
    _, (losses, precs) = run_cores(inputs, trace=False)
    return losses, precs
